# revision 3
# baseline (speedup 1.0000x reference)
"""BinaryFactoredLinear Trainium2 kernel.

Computes out = ((x * s2) @ sign(V)) @ sign(U).T * s1 + bias for
x [4, 4096, 4096] f32, factors [4096, 128] / [4096] — token-sharded
across 8 NeuronCores (2048 tokens each), run SPMD via
run_bass_kernel_spmd.

Host prep (exact f32 math, negligible vs HW time): x2 = x * s2 (same
op order as the reference), then x2 is split into xhi = bf16(x2) and
xlo = bf16(x2 - xhi) — together they carry ~16 mantissa bits, and the
sign matrices are +-1 so bf16 weights are exact. Each core's token
shard is pre-transposed and pre-tiled into contiguous [128, T] blocks
so every DMA is a contiguous 1 MiB transfer with the contraction dim
on SBUF partitions (no on-chip transposes, no on-chip dtype
conversions). The core writes its output transposed as contiguous
[nt, no, 128, T] blocks which the host reassembles.

Per-core pipeline (tokens tiled by T=512, all matmuls N=512 bf16):
  stage 1: z1T[r=128, T] += V_sign_k.T @ xhi_k + V_sign_k.T @ xlo_k
           (32 k-chunks accumulated in one PSUM bank)
  z1 split: DVE re-splits z1 (f32 PSUM) into bf16 hi/lo
  stage 2: outT[o*128:(o+1)*128, T] = U_sign_o @ [z1hi; z1lo]
  epilogue: ScalarE activation(Identity, scale=s1, bias=bias) — both
            per-partition APs — during the PSUM -> SBUF copy.

End-to-end rel err vs the f32 reference: ~3.5e-6 (HW-verified).
Other modes kept for experiments: f32 (exact, 4 cyc/row), f32r
(1 cyc/row, ~1.3e-4 on HW), bf16 (~2.4e-3), bf16x2 (on-chip hi/lo
split, same numerics as bf16x2h but extra ACT/DVE conversion load).
"""

import os
from contextlib import ExitStack

import numpy as np

import concourse.bacc as bacc
import concourse.mybir as mybir
import concourse.tile as tile
from concourse.bass_utils import run_bass_kernel_spmd

F32 = mybir.dt.float32
F32R = mybir.dt.float32r
BF16 = mybir.dt.bfloat16

B, S, D_IN, D_OUT, R = 4, 4096, 4096, 4096, 128
N_CORES = 8
TOKENS = B * S
TOK_PER_CORE = TOKENS // N_CORES

MODE = os.environ.get("BFL_MODE", "bf16x2h")
T_TILE = int(os.environ.get("BFL_T_TILE", "512"))
DMA_GROUP = int(os.environ.get("BFL_DMA_GROUP", "4"))
EPI = os.environ.get("BFL_EPI", "act")
LO_ENG = os.environ.get("BFL_LO_ENG", "dve")
XBUFS = int(os.environ.get("BFL_XBUFS", "5"))
LAYOUT = os.environ.get("BFL_LAYOUT", "std")


def build_nc(mode=MODE, d_in=D_IN, d_out=D_OUT, r=R, tok=TOK_PER_CORE,
             t_tile=T_TILE, loop=1, dma_group=DMA_GROUP, epi=EPI,
             lo_eng=LO_ENG, xbufs=XBUFS, layout=LAYOUT, probe="full",
             odma=os.environ.get("BFL_ODMA", "spread"), obufs=3, opbufs=4):
    if mode == "lite":
        return build_lite(tok=tok, d_in=d_in, d_out=d_out, r=r, loop=loop)
    assert d_in % 128 == 0 and d_out % 128 == 0 and tok % t_tile == 0
    assert r == 128 and t_tile <= 512
    nk, no, nt = d_in // 128, d_out // 128, tok // t_tile
    g = dma_group
    assert nk % g == 0 and no % g == 0

    if mode == "f32":
        xdt = wdt = F32
    elif mode == "f32r":
        xdt = wdt = F32R
    elif mode == "bf16x2h":
        xdt = wdt = BF16
    else:
        xdt, wdt = F32, BF16

    nc = bacc.Bacc("TRN2", target_bir_lowering=False, debug=False)

    if layout == "fat":
        xt = nc.dram_tensor("xt", [nt, nk // g, 128, g, t_tile], xdt,
                            kind="ExternalInput")
        outt = nc.dram_tensor("outt", [nt, no // g, 128, g, t_tile], F32,
                              kind="ExternalOutput")
    else:
        xt = nc.dram_tensor("xt", [nt, nk, 128, t_tile], xdt,
                            kind="ExternalInput")
        outt = nc.dram_tensor("outt", [nt, no, 128, t_tile], F32,
                              kind="ExternalOutput")
    if mode == "bf16x2h":
        assert layout == "std"
        xt2 = nc.dram_tensor("xt2", [nt, nk, 128, t_tile], BF16,
                             kind="ExternalInput")
    w1 = nc.dram_tensor("w1", [128, nk, r], wdt, kind="ExternalInput")
    w2 = nc.dram_tensor("w2", [r, d_out], wdt, kind="ExternalInput")
    s1c = nc.dram_tensor("s1c", [128, no], F32, kind="ExternalInput")
    biasc = nc.dram_tensor("biasc", [128, no], F32, kind="ExternalInput")

    Copy = mybir.ActivationFunctionType.Copy
    Ident = mybir.ActivationFunctionType.Identity
    sub = mybir.AluOpType.subtract
    mult = mybir.AluOpType.mult
    add = mybir.AluOpType.add
    lo_iface = nc.gpsimd if lo_eng == "pool" else nc.vector
    if odma == "spread":
        _rr = [0]

        def _dma():
            _rr[0] += 1
            return nc.sync if _rr[0] % 2 else nc.gpsimd
        in_dma = out_dma = lambda: _dma()
    else:
        out_iface = nc.gpsimd if odma == "pool" else nc.sync
        in_dma = lambda: nc.sync
        out_dma = lambda: out_iface

    with tile.TileContext(nc) as tc, ExitStack() as ctx:
        const = ctx.enter_context(tc.tile_pool(name="const", bufs=1))
        xpool = ctx.enter_context(tc.tile_pool(name="x", bufs=xbufs))
        z1s = ctx.enter_context(tc.tile_pool(name="z1s", bufs=2))
        osb = ctx.enter_context(tc.tile_pool(name="osb", bufs=obufs))
        z1pool = ctx.enter_context(
            tc.tile_pool(name="z1p", bufs=2, space="PSUM"))
        opsum = ctx.enter_context(
            tc.tile_pool(name="opsum", bufs=opbufs, space="PSUM"))
        if mode in ("bf16", "bf16x2"):
            hpool = ctx.enter_context(tc.tile_pool(name="hi", bufs=2 * xbufs))
        if mode == "bf16x2":
            lpool = ctx.enter_context(tc.tile_pool(name="lo", bufs=2 * xbufs))

        w1_sb = const.tile([128, nk, r], wdt)
        nc.sync.dma_start(w1_sb[:], w1.ap())
        w2_sb = const.tile([128, d_out], wdt)
        nc.sync.dma_start(w2_sb[:], w2.ap())
        s1_sb = const.tile([128, no], F32)
        nc.sync.dma_start(s1_sb[:], s1c.ap())
        b_sb = const.tile([128, no], F32)
        nc.sync.dma_start(b_sb[:], biasc.ap())

        if loop > 1:
            loop_cm = tc.For_i(
                0, loop, 1,
                hint_engines=(mybir.EngineType.PE, mybir.EngineType.DVE,
                              mybir.EngineType.Activation,
                              mybir.EngineType.SP))
            ctx.enter_context(loop_cm)

        for t in range(nt):
            z1p = z1pool.tile([128, t_tile], F32)
            xg, xg2 = {}, {}
            for kg in range(nk // g):
                xk = xpool.tile([128, g, t_tile], xdt)
                if probe != "nodma":
                    if layout == "fat":
                        in_dma().dma_start(xk[:], xt.ap()[t, kg])
                    else:
                        in_dma().dma_start(
                            xk[:], xt.ap()[t, kg * g:(kg + 1) * g].rearrange(
                                "g p s -> p g s"))
                xg[kg] = xk
                if mode == "bf16x2h":
                    xk2 = xpool.tile([128, g, t_tile], BF16, tag="xk2",
                                     name="xk2")
                    if probe != "nodma":
                        in_dma().dma_start(
                            xk2[:],
                            xt2.ap()[t, kg * g:(kg + 1) * g].rearrange(
                                "g p s -> p g s"))
                    xg2[kg] = xk2
            for k in range(nk):
                xk = xg[k // g][:, k % g, :]
                first, last = k == 0, k == nk - 1
                if mode == "bf16x2h":
                    xk2 = xg2[k // g][:, k % g, :]
                    nc.tensor.matmul(z1p[:], w1_sb[:, k, :], xk,
                                     start=first, stop=False)
                    nc.tensor.matmul(z1p[:], w1_sb[:, k, :], xk2,
                                     start=False, stop=last)
                elif mode in ("bf16", "bf16x2"):
                    hi = hpool.tile([128, t_tile], BF16)
                    nc.scalar.activation(hi[:], xk, Copy)
                    if mode == "bf16x2":
                        lo = lpool.tile([128, t_tile], BF16)
                        lo_iface.tensor_tensor(lo[:], xk, hi[:], sub)
                        nc.tensor.matmul(z1p[:], w1_sb[:, k, :], hi[:],
                                         start=first, stop=False)
                        nc.tensor.matmul(z1p[:], w1_sb[:, k, :], lo[:],
                                         start=False, stop=last)
                    else:
                        nc.tensor.matmul(z1p[:], w1_sb[:, k, :], hi[:],
                                         start=first, stop=last)
                else:
                    nc.tensor.matmul(z1p[:], w1_sb[:, k, :], xk,
                                     start=first, stop=last)

            if mode in ("bf16", "bf16x2", "bf16x2h"):
                z1hi = z1s.tile([128, t_tile], BF16, tag="z1hi")
                nc.vector.tensor_copy(z1hi[:], z1p[:])
                movers = [z1hi]
                if mode in ("bf16x2", "bf16x2h"):
                    z1lo = z1s.tile([128, t_tile], BF16, tag="z1lo")
                    nc.vector.tensor_tensor(z1lo[:], z1p[:], z1hi[:], sub)
                    movers.append(z1lo)
            else:
                z1f = z1s.tile([128, t_tile], xdt, tag="z1f")
                nc.vector.tensor_copy(z1f[:], z1p[:])
                movers = [z1f]

            for og in range(no // g):
                ob = osb.tile([128, g, t_tile], F32)
                for oi in range(g):
                    o = og * g + oi
                    op = opsum.tile([128, t_tile], F32)
                    for i, mv in enumerate(movers):
                        nc.tensor.matmul(
                            op[:], w2_sb[:, o * 128:(o + 1) * 128], mv[:],
                            start=(i == 0), stop=(i == len(movers) - 1))
                    if epi == "act":
                        nc.scalar.activation(ob[:, oi, :], op[:], Ident,
                                             bias=b_sb[:, o:o + 1],
                                             scale=s1_sb[:, o:o + 1])
                    else:
                        nc.vector.tensor_scalar(ob[:, oi, :], op[:],
                                                s1_sb[:, o:o + 1],
                                                b_sb[:, o:o + 1], mult, add)
                if probe != "nodma":
                    if layout == "fat":
                        out_dma().dma_start(outt.ap()[t, og], ob[:])
                    else:
                        out_dma().dma_start(
                            outt.ap()[t, og * g:(og + 1) * g].rearrange(
                                "g p s -> p g s"), ob[:])

    nc.compile()
    return nc


def build_lite(tok=TOK_PER_CORE, d_in=D_IN, d_out=D_OUT, r=R, loop=1,
               g_in=int(os.environ.get("BFL_GIN", "4")),
               g_out=int(os.environ.get("BFL_GOUT", "2")),
               xbufs=int(os.environ.get("BFL_LXBUFS", "3")),
               obufs=int(os.environ.get("BFL_LOBUFS", "3")),
               opbufs=4,
               in_eng=os.environ.get("BFL_LIN", "sync"),
               out_eng=os.environ.get("BFL_LOUT", "act")):
    """Memory-lean variant: bf16 x (s2 folded on host), bf16 sign(V),
    f32r stage-2, bf16 output. Per-core HBM traffic ~35 MB vs ~69 MB
    for bf16x2h. x is stored as [nk, 128, tok] k-planes so every input
    DMA is g_in contiguous 512 KiB planes; output is [no, 128, tok]
    o-planes, g_out planes per DMA."""
    assert d_in % 128 == 0 and d_out % 128 == 0 and r == 128
    nk, no, nt = d_in // 128, d_out // 128, tok // 512
    assert nk % g_in == 0 and no % g_out == 0

    nc = bacc.Bacc("TRN2", target_bir_lowering=False, debug=False)
    xt = nc.dram_tensor("xt", [nk, 128, tok], BF16, kind="ExternalInput")
    outt = nc.dram_tensor("outt", [no, 128, tok], BF16,
                          kind="ExternalOutput")
    w1 = nc.dram_tensor("w1", [128, nk, r], BF16, kind="ExternalInput")
    w2 = nc.dram_tensor("w2", [r, d_out], F32R, kind="ExternalInput")
    s1c = nc.dram_tensor("s1c", [128, no], F32, kind="ExternalInput")
    biasc = nc.dram_tensor("biasc", [128, no], F32, kind="ExternalInput")

    Ident = mybir.ActivationFunctionType.Identity
    eng = {"sync": nc.sync, "act": nc.scalar, "pool": nc.gpsimd}
    in_dma, out_dma = eng[in_eng], eng[out_eng]

    with tile.TileContext(nc) as tc, ExitStack() as ctx:
        const = ctx.enter_context(tc.tile_pool(name="const", bufs=1))
        xpool = ctx.enter_context(tc.tile_pool(name="x", bufs=xbufs))
        z1s = ctx.enter_context(tc.tile_pool(name="z1s", bufs=2 * nt))
        osb = ctx.enter_context(tc.tile_pool(name="osb", bufs=obufs))
        z1pool = ctx.enter_context(
            tc.tile_pool(name="z1p", bufs=nt, space="PSUM"))
        opsum = ctx.enter_context(
            tc.tile_pool(name="opsum", bufs=opbufs, space="PSUM"))

        w1_sb = const.tile([128, nk, r], BF16)
        nc.sync.dma_start(w1_sb[:], w1.ap())
        w2_sb = const.tile([128, d_out], F32R)
        nc.sync.dma_start(w2_sb[:], w2.ap())
        s1_sb = const.tile([128, no], F32)
        nc.sync.dma_start(s1_sb[:], s1c.ap())
        b_sb = const.tile([128, no], F32)
        nc.sync.dma_start(b_sb[:], biasc.ap())

        if loop > 1:
            loop_cm = tc.For_i(
                0, loop, 1,
                hint_engines=(mybir.EngineType.PE, mybir.EngineType.DVE,
                              mybir.EngineType.Activation,
                              mybir.EngineType.SP))
            ctx.enter_context(loop_cm)

        xg = {}
        for kg in range(nk // g_in):
            xk = xpool.tile([128, g_in, tok], BF16)
            in_dma.dma_start(
                xk[:], xt.ap()[kg * g_in:(kg + 1) * g_in].rearrange(
                    "g p s -> p g s"))
            xg[kg] = xk
        z1p = [z1pool.tile([128, 512], F32, tag=f"z1p{t}")
               for t in range(nt)]
        for k in range(nk):
            xk = xg[k // g_in][:, k % g_in, :]
            first, last = k == 0, k == nk - 1
            for t in range(nt):
                nc.tensor.matmul(z1p[t][:], w1_sb[:, k, :],
                                 xk[:, t * 512:(t + 1) * 512],
                                 start=first, stop=last)
        z1r = []
        for t in range(nt):
            zr = z1s.tile([128, 512], F32R, tag=f"z1r{t}")
            nc.vector.tensor_copy(zr[:], z1p[t][:])
            z1r.append(zr)
        for og in range(no // g_out):
            ob = osb.tile([128, g_out, tok], BF16)
            for oi in range(g_out):
                o = og * g_out + oi
                for t in range(nt):
                    op = opsum.tile([128, 512], F32)
                    nc.tensor.matmul(op[:], w2_sb[:, o * 128:(o + 1) * 128],
                                     z1r[t][:], start=True, stop=True)
                    nc.scalar.activation(ob[:, oi, t * 512:(t + 1) * 512],
                                         op[:], Ident,
                                         bias=b_sb[:, o:o + 1],
                                         scale=s1_sb[:, o:o + 1])
            out_dma.dma_start(
                outt.ap()[og * g_out:(og + 1) * g_out].rearrange(
                    "g p s -> p g s"), ob[:])

    nc.compile()
    return nc


def prep_lite(x, U_latent, V_latent, s1, s2, bias, n_cores=N_CORES):
    import ml_dtypes

    tokens = x.shape[0] * x.shape[1] if x.ndim == 3 else x.shape[0]
    d_in = x.shape[-1]
    tok_pc = tokens // n_cores
    nk = d_in // 128

    x2 = x.reshape(tokens, d_in) * s2[None, :]
    xh = x2.astype(ml_dtypes.bfloat16)
    w1 = np.sign(V_latent).astype(np.float32)
    w1 = np.ascontiguousarray(
        w1.reshape(nk, 128, -1).transpose(1, 0, 2)).astype(
            ml_dtypes.bfloat16)
    # f32r has fp32 bit layout; pass the sign matrix as plain float32
    w2 = np.ascontiguousarray(np.sign(U_latent).astype(np.float32).T)
    no = w2.shape[1] // 128
    s1c = np.ascontiguousarray(s1.reshape(no, 128).T)
    biasc = np.ascontiguousarray(bias.reshape(no, 128).T)

    in_maps = []
    for c in range(n_cores):
        xs = xh[c * tok_pc:(c + 1) * tok_pc]
        xtc = np.ascontiguousarray(
            xs.reshape(tok_pc, nk, 128).transpose(1, 2, 0))
        in_maps.append({"xt": xtc, "w1": w1, "w2": w2, "s1c": s1c,
                        "biasc": biasc})
    return in_maps


def gather_lite(results, n_cores=N_CORES):
    out = np.empty((TOKENS, D_OUT), np.float32)
    for c in range(n_cores):
        ot = results[c]["outt"]  # [no, 128, tok_pc] bf16
        out[c * TOK_PER_CORE:(c + 1) * TOK_PER_CORE] = (
            ot.transpose(2, 0, 1).reshape(TOK_PER_CORE, D_OUT).astype(
                np.float32))
    return out.reshape(B, S, D_OUT)


def prep_inputs(x, U_latent, V_latent, s1, s2, bias, mode=MODE,
                n_cores=N_CORES, t_tile=T_TILE, layout=LAYOUT,
                dma_group=DMA_GROUP):
    if mode == "lite":
        return prep_lite(x, U_latent, V_latent, s1, s2, bias,
                         n_cores=n_cores)
    """Host-side prep: fold s2 into x, sign + cast factors, shard tokens."""
    import ml_dtypes

    tokens = x.shape[0] * x.shape[1] if x.ndim == 3 else x.shape[0]
    d_in = x.shape[-1]
    tok_pc = tokens // n_cores
    nt, nk = tok_pc // t_tile, d_in // 128
    g = dma_group

    x2 = x.reshape(tokens, d_in) * s2[None, :]
    w1 = np.sign(V_latent).astype(np.float32)
    # pack [d_in, r] -> [128, nk, r] so the SBUF upload is contiguous
    w1 = np.ascontiguousarray(
        w1.reshape(nk, 128, -1).transpose(1, 0, 2))
    w2 = np.ascontiguousarray(np.sign(U_latent).astype(np.float32).T)
    if mode in ("bf16", "bf16x2", "bf16x2h"):
        w1 = w1.astype(ml_dtypes.bfloat16)
        w2 = w2.astype(ml_dtypes.bfloat16)
    if mode == "bf16x2h":
        xhi = x2.astype(ml_dtypes.bfloat16)
        xlo = (x2 - xhi.astype(np.float32)).astype(ml_dtypes.bfloat16)
    no = w2.shape[1] // 128
    s1c = np.ascontiguousarray(s1.reshape(no, 128).T)
    biasc = np.ascontiguousarray(bias.reshape(no, 128).T)

    def tilefmt(arr2d, c):
        xs = arr2d[c * tok_pc:(c + 1) * tok_pc, :]
        if layout == "fat":
            # [nt, T, nk/g, g, 128] -> [nt, nk/g, 128, g, T]:
            # per partition a contiguous g*T run
            return np.ascontiguousarray(
                xs.reshape(nt, t_tile, nk // g, g, 128).transpose(
                    0, 2, 4, 3, 1))
        # [nt, T, nk, 128] -> [nt, nk, 128, T]
        return np.ascontiguousarray(
            xs.reshape(nt, t_tile, nk, 128).transpose(0, 2, 3, 1))

    in_maps = []
    for c in range(n_cores):
        m = {"w1": w1, "w2": w2, "s1c": s1c, "biasc": biasc}
        if mode == "bf16x2h":
            m["xt"] = tilefmt(xhi, c)
            m["xt2"] = tilefmt(xlo, c)
        else:
            m["xt"] = tilefmt(x2, c)
        in_maps.append(m)
    return in_maps


def gather_out(results, n_cores=N_CORES, t_tile=T_TILE, layout=LAYOUT,
               dma_group=DMA_GROUP):
    out = np.empty((TOKENS, D_OUT), np.float32)
    for c in range(n_cores):
        ot = results[c]["outt"]
        if layout == "fat":
            # [nt, no/g, 128, g, T] -> [tok_pc, d_out]
            shard = ot.transpose(0, 4, 1, 3, 2).reshape(TOK_PER_CORE, D_OUT)
        else:
            # [nt, no, 128, T] -> [tok_pc, d_out]
            shard = ot.transpose(0, 3, 1, 2).reshape(TOK_PER_CORE, D_OUT)
        out[c * TOK_PER_CORE:(c + 1) * TOK_PER_CORE, :] = shard
    return out.reshape(B, S, D_OUT)


_NC_CACHE = {}


def run(inputs, mode=MODE, trace=False):
    if mode not in _NC_CACHE:
        _NC_CACHE[mode] = build_nc(mode=mode)
    nc = _NC_CACHE[mode]
    in_maps = prep_inputs(**inputs, mode=mode)
    res = run_bass_kernel_spmd(nc, in_maps, list(range(N_CORES)),
                               trace=trace)
    return gather_out(res.results), res


def kernel(**inputs):
    inputs = {k: np.asarray(v) for k, v in inputs.items()}
    out, _ = run(inputs)
    return out



# revision 11
# speedup vs baseline: 1.5903x; 1.5903x over previous
"""BinaryFactoredLinear Trainium2 kernel.

Computes out = ((x * s2) @ sign(V)) @ sign(U).T * s1 + bias for
x [4, 4096, 4096] f32, factors [4096, 128] / [4096] — token-sharded
across 8 NeuronCores (2048 tokens each), run SPMD via
run_bass_kernel_spmd.

Host prep (exact f32 math, negligible vs HW time): x2 = x * s2 (same
op order as the reference), then x2 is split into xhi = bf16(x2) and
xlo = bf16(x2 - xhi) — together they carry ~16 mantissa bits, and the
sign matrices are +-1 so bf16 weights are exact. Each core's token
shard is pre-transposed and pre-tiled into contiguous [128, T] blocks
so every DMA is a contiguous 1 MiB transfer with the contraction dim
on SBUF partitions (no on-chip transposes, no on-chip dtype
conversions). The core writes its output transposed as contiguous
[nt, no, 128, T] blocks which the host reassembles.

Per-core pipeline (tokens tiled by T=512, all matmuls N=512 bf16):
  stage 1: z1T[r=128, T] += V_sign_k.T @ xhi_k + V_sign_k.T @ xlo_k
           (32 k-chunks accumulated in one PSUM bank)
  z1 split: DVE re-splits z1 (f32 PSUM) into bf16 hi/lo
  stage 2: outT[o*128:(o+1)*128, T] = U_sign_o @ [z1hi; z1lo]
  epilogue: ScalarE activation(Identity, scale=s1, bias=bias) — both
            per-partition APs — during the PSUM -> SBUF copy.

End-to-end rel err vs the f32 reference: ~3.5e-6 (HW-verified).
Other modes kept for experiments: f32 (exact, 4 cyc/row), f32r
(1 cyc/row, ~1.3e-4 on HW), bf16 (~2.4e-3), bf16x2 (on-chip hi/lo
split, same numerics as bf16x2h but extra ACT/DVE conversion load).
"""

import os
from contextlib import ExitStack

import numpy as np

import concourse.bacc as bacc
import concourse.mybir as mybir
import concourse.tile as tile
from concourse.bass_utils import run_bass_kernel_spmd

F32 = mybir.dt.float32
F32R = mybir.dt.float32r
BF16 = mybir.dt.bfloat16

B, S, D_IN, D_OUT, R = 4, 4096, 4096, 4096, 128
N_CORES = 8
TOKENS = B * S
TOK_PER_CORE = TOKENS // N_CORES

MODE = os.environ.get("BFL_MODE", "bf16x2h")
T_TILE = int(os.environ.get("BFL_T_TILE", "512"))
DMA_GROUP = int(os.environ.get("BFL_DMA_GROUP", "4"))
EPI = os.environ.get("BFL_EPI", "act")
LO_ENG = os.environ.get("BFL_LO_ENG", "dve")
XBUFS = int(os.environ.get("BFL_XBUFS", "5"))
LAYOUT = os.environ.get("BFL_LAYOUT", "std")


def build_nc(mode=MODE, d_in=D_IN, d_out=D_OUT, r=R, tok=TOK_PER_CORE,
             t_tile=T_TILE, loop=1, dma_group=DMA_GROUP, epi=EPI,
             lo_eng=LO_ENG, xbufs=XBUFS, layout=LAYOUT, probe="full",
             odma=os.environ.get("BFL_ODMA", "spread"), obufs=3, opbufs=4):
    if mode == "lite":
        return build_lite(tok=tok, d_in=d_in, d_out=d_out, r=r, loop=loop)
    assert d_in % 128 == 0 and d_out % 128 == 0 and tok % t_tile == 0
    assert r == 128 and t_tile <= 512
    nk, no, nt = d_in // 128, d_out // 128, tok // t_tile
    g = dma_group
    assert nk % g == 0 and no % g == 0

    if mode == "f32":
        xdt = wdt = F32
    elif mode == "f32r":
        xdt = wdt = F32R
    elif mode == "bf16x2h":
        xdt = wdt = BF16
    else:
        xdt, wdt = F32, BF16

    nc = bacc.Bacc("TRN2", target_bir_lowering=False, debug=False)

    if layout == "fat":
        xt = nc.dram_tensor("xt", [nt, nk // g, 128, g, t_tile], xdt,
                            kind="ExternalInput")
        outt = nc.dram_tensor("outt", [nt, no // g, 128, g, t_tile], F32,
                              kind="ExternalOutput")
    else:
        xt = nc.dram_tensor("xt", [nt, nk, 128, t_tile], xdt,
                            kind="ExternalInput")
        outt = nc.dram_tensor("outt", [nt, no, 128, t_tile], F32,
                              kind="ExternalOutput")
    if mode == "bf16x2h":
        assert layout == "std"
        xt2 = nc.dram_tensor("xt2", [nt, nk, 128, t_tile], BF16,
                             kind="ExternalInput")
    w1 = nc.dram_tensor("w1", [128, nk, r], wdt, kind="ExternalInput")
    w2 = nc.dram_tensor("w2", [r, d_out], wdt, kind="ExternalInput")
    s1c = nc.dram_tensor("s1c", [128, no], F32, kind="ExternalInput")
    biasc = nc.dram_tensor("biasc", [128, no], F32, kind="ExternalInput")

    Copy = mybir.ActivationFunctionType.Copy
    Ident = mybir.ActivationFunctionType.Identity
    sub = mybir.AluOpType.subtract
    mult = mybir.AluOpType.mult
    add = mybir.AluOpType.add
    lo_iface = nc.gpsimd if lo_eng == "pool" else nc.vector
    if odma == "spread":
        _rr = [0]

        def _dma():
            _rr[0] += 1
            return nc.sync if _rr[0] % 2 else nc.gpsimd
        in_dma = out_dma = lambda: _dma()
    else:
        out_iface = nc.gpsimd if odma == "pool" else nc.sync
        in_dma = lambda: nc.sync
        out_dma = lambda: out_iface

    with tile.TileContext(nc) as tc, ExitStack() as ctx:
        const = ctx.enter_context(tc.tile_pool(name="const", bufs=1))
        xpool = ctx.enter_context(tc.tile_pool(name="x", bufs=xbufs))
        z1s = ctx.enter_context(tc.tile_pool(name="z1s", bufs=2))
        osb = ctx.enter_context(tc.tile_pool(name="osb", bufs=obufs))
        z1pool = ctx.enter_context(
            tc.tile_pool(name="z1p", bufs=2, space="PSUM"))
        opsum = ctx.enter_context(
            tc.tile_pool(name="opsum", bufs=opbufs, space="PSUM"))
        if mode in ("bf16", "bf16x2"):
            hpool = ctx.enter_context(tc.tile_pool(name="hi", bufs=2 * xbufs))
        if mode == "bf16x2":
            lpool = ctx.enter_context(tc.tile_pool(name="lo", bufs=2 * xbufs))

        w1_sb = const.tile([128, nk, r], wdt)
        nc.sync.dma_start(w1_sb[:], w1.ap())
        w2_sb = const.tile([128, d_out], wdt)
        nc.sync.dma_start(w2_sb[:], w2.ap())
        s1_sb = const.tile([128, no], F32)
        nc.sync.dma_start(s1_sb[:], s1c.ap())
        b_sb = const.tile([128, no], F32)
        nc.sync.dma_start(b_sb[:], biasc.ap())

        if loop > 1:
            loop_cm = tc.For_i(
                0, loop, 1,
                hint_engines=(mybir.EngineType.PE, mybir.EngineType.DVE,
                              mybir.EngineType.Activation,
                              mybir.EngineType.SP))
            ctx.enter_context(loop_cm)

        for t in range(nt):
            z1p = z1pool.tile([128, t_tile], F32)
            xg, xg2 = {}, {}
            for kg in range(nk // g):
                xk = xpool.tile([128, g, t_tile], xdt)
                if probe != "nodma":
                    if layout == "fat":
                        in_dma().dma_start(xk[:], xt.ap()[t, kg])
                    else:
                        in_dma().dma_start(
                            xk[:], xt.ap()[t, kg * g:(kg + 1) * g].rearrange(
                                "g p s -> p g s"))
                xg[kg] = xk
                if mode == "bf16x2h":
                    xk2 = xpool.tile([128, g, t_tile], BF16, tag="xk2",
                                     name="xk2")
                    if probe != "nodma":
                        in_dma().dma_start(
                            xk2[:],
                            xt2.ap()[t, kg * g:(kg + 1) * g].rearrange(
                                "g p s -> p g s"))
                    xg2[kg] = xk2
            for k in range(nk):
                xk = xg[k // g][:, k % g, :]
                first, last = k == 0, k == nk - 1
                if mode == "bf16x2h":
                    xk2 = xg2[k // g][:, k % g, :]
                    nc.tensor.matmul(z1p[:], w1_sb[:, k, :], xk,
                                     start=first, stop=False)
                    nc.tensor.matmul(z1p[:], w1_sb[:, k, :], xk2,
                                     start=False, stop=last)
                elif mode in ("bf16", "bf16x2"):
                    hi = hpool.tile([128, t_tile], BF16)
                    nc.scalar.activation(hi[:], xk, Copy)
                    if mode == "bf16x2":
                        lo = lpool.tile([128, t_tile], BF16)
                        lo_iface.tensor_tensor(lo[:], xk, hi[:], sub)
                        nc.tensor.matmul(z1p[:], w1_sb[:, k, :], hi[:],
                                         start=first, stop=False)
                        nc.tensor.matmul(z1p[:], w1_sb[:, k, :], lo[:],
                                         start=False, stop=last)
                    else:
                        nc.tensor.matmul(z1p[:], w1_sb[:, k, :], hi[:],
                                         start=first, stop=last)
                else:
                    nc.tensor.matmul(z1p[:], w1_sb[:, k, :], xk,
                                     start=first, stop=last)

            if mode in ("bf16", "bf16x2", "bf16x2h"):
                z1hi = z1s.tile([128, t_tile], BF16, tag="z1hi")
                nc.vector.tensor_copy(z1hi[:], z1p[:])
                movers = [z1hi]
                if mode in ("bf16x2", "bf16x2h"):
                    z1lo = z1s.tile([128, t_tile], BF16, tag="z1lo")
                    nc.vector.tensor_tensor(z1lo[:], z1p[:], z1hi[:], sub)
                    movers.append(z1lo)
            else:
                z1f = z1s.tile([128, t_tile], xdt, tag="z1f")
                nc.vector.tensor_copy(z1f[:], z1p[:])
                movers = [z1f]

            for og in range(no // g):
                ob = osb.tile([128, g, t_tile], F32)
                for oi in range(g):
                    o = og * g + oi
                    op = opsum.tile([128, t_tile], F32)
                    for i, mv in enumerate(movers):
                        nc.tensor.matmul(
                            op[:], w2_sb[:, o * 128:(o + 1) * 128], mv[:],
                            start=(i == 0), stop=(i == len(movers) - 1))
                    if epi == "act":
                        nc.scalar.activation(ob[:, oi, :], op[:], Ident,
                                             bias=b_sb[:, o:o + 1],
                                             scale=s1_sb[:, o:o + 1])
                    else:
                        nc.vector.tensor_scalar(ob[:, oi, :], op[:],
                                                s1_sb[:, o:o + 1],
                                                b_sb[:, o:o + 1], mult, add)
                if probe != "nodma":
                    if layout == "fat":
                        out_dma().dma_start(outt.ap()[t, og], ob[:])
                    else:
                        out_dma().dma_start(
                            outt.ap()[t, og * g:(og + 1) * g].rearrange(
                                "g p s -> p g s"), ob[:])

    nc.compile()
    return nc


def build_lite(tok=TOK_PER_CORE, d_in=D_IN, d_out=D_OUT, r=R, loop=1,
               g_in=int(os.environ.get("BFL_GIN", "4")),
               g_out=int(os.environ.get("BFL_GOUT", "2")),
               xbufs=int(os.environ.get("BFL_LXBUFS", "3")),
               obufs=int(os.environ.get("BFL_LOBUFS", "3")),
               opbufs=4,
               in_eng=os.environ.get("BFL_LIN", "sync"),
               out_eng=os.environ.get("BFL_LOUT", "act"),
               lay=os.environ.get("BFL_LLAYOUT", "plane")):
    """Memory-lean variant: bf16 x (s2 folded on host), bf16 sign(V),
    f32r stage-2, bf16 output. Per-core HBM traffic ~35 MB vs ~69 MB
    for bf16x2h. x is stored as [nk, 128, tok] k-planes so every input
    DMA is g_in contiguous 512 KiB planes; output is [no, 128, tok]
    o-planes, g_out planes per DMA."""
    assert d_in % 128 == 0 and d_out % 128 == 0 and r == 128
    nk, no, nt = d_in // 128, d_out // 128, tok // 512
    assert nk % g_in == 0 and no % g_out == 0

    nc = bacc.Bacc("TRN2", target_bir_lowering=False, debug=False)
    if lay == "fat":
        xt = nc.dram_tensor("xt", [nk // g_in, 128, g_in, tok], BF16,
                            kind="ExternalInput")
        outt = nc.dram_tensor("outt", [no // g_out, 128, g_out, tok], BF16,
                              kind="ExternalOutput")
    else:
        xt = nc.dram_tensor("xt", [nk, 128, tok], BF16,
                            kind="ExternalInput")
        outt = nc.dram_tensor("outt", [no, 128, tok], BF16,
                              kind="ExternalOutput")
    w1 = nc.dram_tensor("w1", [128, nk, r], BF16, kind="ExternalInput")
    w2 = nc.dram_tensor("w2", [r, d_out], F32R, kind="ExternalInput")
    s1c = nc.dram_tensor("s1c", [128, no], F32, kind="ExternalInput")
    biasc = nc.dram_tensor("biasc", [128, no], F32, kind="ExternalInput")

    Ident = mybir.ActivationFunctionType.Identity
    eng = {"sync": nc.sync, "act": nc.scalar, "pool": nc.gpsimd}
    in_dma, out_dma = eng[in_eng], eng[out_eng]

    with tile.TileContext(nc) as tc, ExitStack() as ctx:
        const = ctx.enter_context(tc.tile_pool(name="const", bufs=1))
        xpool = ctx.enter_context(tc.tile_pool(name="x", bufs=xbufs))
        z1s = ctx.enter_context(tc.tile_pool(name="z1s", bufs=2))
        osb = ctx.enter_context(tc.tile_pool(name="osb", bufs=obufs))
        z1pool = ctx.enter_context(
            tc.tile_pool(name="z1p", bufs=1, space="PSUM"))
        opsum = ctx.enter_context(
            tc.tile_pool(name="opsum", bufs=opbufs, space="PSUM"))

        w1_sb = const.tile([128, nk, r], BF16)
        nc.sync.dma_start(w1_sb[:], w1.ap())
        w2_sb = const.tile([128, d_out], F32R)
        nc.sync.dma_start(w2_sb[:], w2.ap())
        s1_sb = const.tile([128, no], F32)
        nc.sync.dma_start(s1_sb[:], s1c.ap())
        b_sb = const.tile([128, no], F32)
        nc.sync.dma_start(b_sb[:], biasc.ap())

        if loop > 1:
            loop_cm = tc.For_i(
                0, loop, 1,
                hint_engines=(mybir.EngineType.PE, mybir.EngineType.DVE,
                              mybir.EngineType.Activation,
                              mybir.EngineType.SP))
            ctx.enter_context(loop_cm)

        xg = {}
        for kg in range(nk // g_in):
            xk = xpool.tile([128, g_in, tok], BF16)
            if lay == "fat":
                in_dma.dma_start(xk[:], xt.ap()[kg])
            else:
                in_dma.dma_start(
                    xk[:], xt.ap()[kg * g_in:(kg + 1) * g_in].rearrange(
                        "g p s -> p g s"))
            xg[kg] = xk
        z1p = [z1pool.tile([128, 512], F32, tag=f"z1p{t}", name=f"z1p{t}")
               for t in range(nt)]
        for k in range(nk):
            xk = xg[k // g_in][:, k % g_in, :]
            first, last = k == 0, k == nk - 1
            for t in range(nt):
                nc.tensor.matmul(z1p[t][:], w1_sb[:, k, :],
                                 xk[:, t * 512:(t + 1) * 512],
                                 start=first, stop=last)
        z1r = []
        for t in range(nt):
            zr = z1s.tile([128, 512], F32R, tag=f"z1r{t}")
            nc.vector.tensor_copy(zr[:], z1p[t][:])
            z1r.append(zr)
        for og in range(no // g_out):
            ob = osb.tile([128, g_out, tok], BF16)
            for oi in range(g_out):
                o = og * g_out + oi
                for t in range(nt):
                    op = opsum.tile([128, 512], F32)
                    nc.tensor.matmul(op[:], w2_sb[:, o * 128:(o + 1) * 128],
                                     z1r[t][:], start=True, stop=True)
                    nc.scalar.activation(ob[:, oi, t * 512:(t + 1) * 512],
                                         op[:], Ident,
                                         bias=b_sb[:, o:o + 1],
                                         scale=s1_sb[:, o:o + 1])
            if lay == "fat":
                out_dma.dma_start(outt.ap()[og], ob[:])
            else:
                out_dma.dma_start(
                    outt.ap()[og * g_out:(og + 1) * g_out].rearrange(
                        "g p s -> p g s"), ob[:])

    nc.compile()
    return nc


def prep_lite(x, U_latent, V_latent, s1, s2, bias, n_cores=N_CORES,
              g_in=int(os.environ.get("BFL_GIN", "4")),
              lay=os.environ.get("BFL_LLAYOUT", "plane")):
    import ml_dtypes

    tokens = x.shape[0] * x.shape[1] if x.ndim == 3 else x.shape[0]
    d_in = x.shape[-1]
    tok_pc = tokens // n_cores
    nk = d_in // 128

    x2 = x.reshape(tokens, d_in) * s2[None, :]
    xh = x2.astype(ml_dtypes.bfloat16)
    w1 = np.sign(V_latent).astype(np.float32)
    w1 = np.ascontiguousarray(
        w1.reshape(nk, 128, -1).transpose(1, 0, 2)).astype(
            ml_dtypes.bfloat16)
    # f32r has fp32 bit layout; pass the sign matrix as plain float32
    w2 = np.ascontiguousarray(np.sign(U_latent).astype(np.float32).T)
    no = w2.shape[1] // 128
    s1c = np.ascontiguousarray(s1.reshape(no, 128).T)
    biasc = np.ascontiguousarray(bias.reshape(no, 128).T)

    in_maps = []
    for c in range(n_cores):
        xs = xh[c * tok_pc:(c + 1) * tok_pc]
        if lay == "fat":
            # [nk//g, 128, g, tok]: one contiguous g*tok run per partition
            xtc = np.ascontiguousarray(
                xs.reshape(tok_pc, nk // g_in, g_in, 128).transpose(
                    1, 3, 2, 0))
        else:
            xtc = np.ascontiguousarray(
                xs.reshape(tok_pc, nk, 128).transpose(1, 2, 0))
        in_maps.append({"xt": xtc, "w1": w1, "w2": w2, "s1c": s1c,
                        "biasc": biasc})
    return in_maps


def gather_lite(results, n_cores=N_CORES,
                lay=os.environ.get("BFL_LLAYOUT", "plane")):
    out = np.empty((TOKENS, D_OUT), np.float32)
    for c in range(n_cores):
        ot = results[c]["outt"]
        if lay == "fat":
            # [no//g, 128, g, tok] -> [tok, d_out]
            shard = ot.transpose(3, 0, 2, 1).reshape(TOK_PER_CORE, D_OUT)
        else:
            # [no, 128, tok] -> [tok, d_out]
            shard = ot.transpose(2, 0, 1).reshape(TOK_PER_CORE, D_OUT)
        out[c * TOK_PER_CORE:(c + 1) * TOK_PER_CORE] = shard.astype(
            np.float32)
    return out.reshape(B, S, D_OUT)


def prep_inputs(x, U_latent, V_latent, s1, s2, bias, mode=MODE,
                n_cores=N_CORES, t_tile=T_TILE, layout=LAYOUT,
                dma_group=DMA_GROUP):
    if mode == "lite":
        return prep_lite(x, U_latent, V_latent, s1, s2, bias,
                         n_cores=n_cores)
    """Host-side prep: fold s2 into x, sign + cast factors, shard tokens."""
    import ml_dtypes

    tokens = x.shape[0] * x.shape[1] if x.ndim == 3 else x.shape[0]
    d_in = x.shape[-1]
    tok_pc = tokens // n_cores
    nt, nk = tok_pc // t_tile, d_in // 128
    g = dma_group

    x2 = x.reshape(tokens, d_in) * s2[None, :]
    w1 = np.sign(V_latent).astype(np.float32)
    # pack [d_in, r] -> [128, nk, r] so the SBUF upload is contiguous
    w1 = np.ascontiguousarray(
        w1.reshape(nk, 128, -1).transpose(1, 0, 2))
    w2 = np.ascontiguousarray(np.sign(U_latent).astype(np.float32).T)
    if mode in ("bf16", "bf16x2", "bf16x2h"):
        w1 = w1.astype(ml_dtypes.bfloat16)
        w2 = w2.astype(ml_dtypes.bfloat16)
    if mode == "bf16x2h":
        xhi = x2.astype(ml_dtypes.bfloat16)
        xlo = (x2 - xhi.astype(np.float32)).astype(ml_dtypes.bfloat16)
    no = w2.shape[1] // 128
    s1c = np.ascontiguousarray(s1.reshape(no, 128).T)
    biasc = np.ascontiguousarray(bias.reshape(no, 128).T)

    def tilefmt(arr2d, c):
        xs = arr2d[c * tok_pc:(c + 1) * tok_pc, :]
        if layout == "fat":
            # [nt, T, nk/g, g, 128] -> [nt, nk/g, 128, g, T]:
            # per partition a contiguous g*T run
            return np.ascontiguousarray(
                xs.reshape(nt, t_tile, nk // g, g, 128).transpose(
                    0, 2, 4, 3, 1))
        # [nt, T, nk, 128] -> [nt, nk, 128, T]
        return np.ascontiguousarray(
            xs.reshape(nt, t_tile, nk, 128).transpose(0, 2, 3, 1))

    in_maps = []
    for c in range(n_cores):
        m = {"w1": w1, "w2": w2, "s1c": s1c, "biasc": biasc}
        if mode == "bf16x2h":
            m["xt"] = tilefmt(xhi, c)
            m["xt2"] = tilefmt(xlo, c)
        else:
            m["xt"] = tilefmt(x2, c)
        in_maps.append(m)
    return in_maps


def gather_out(results, n_cores=N_CORES, t_tile=T_TILE, layout=LAYOUT,
               dma_group=DMA_GROUP):
    out = np.empty((TOKENS, D_OUT), np.float32)
    for c in range(n_cores):
        ot = results[c]["outt"]
        if layout == "fat":
            # [nt, no/g, 128, g, T] -> [tok_pc, d_out]
            shard = ot.transpose(0, 4, 1, 3, 2).reshape(TOK_PER_CORE, D_OUT)
        else:
            # [nt, no, 128, T] -> [tok_pc, d_out]
            shard = ot.transpose(0, 3, 1, 2).reshape(TOK_PER_CORE, D_OUT)
        out[c * TOK_PER_CORE:(c + 1) * TOK_PER_CORE, :] = shard
    return out.reshape(B, S, D_OUT)


_NC_CACHE = {}


def run(inputs, mode=MODE, trace=False):
    if mode not in _NC_CACHE:
        _NC_CACHE[mode] = build_nc(mode=mode)
    nc = _NC_CACHE[mode]
    in_maps = prep_inputs(**inputs, mode=mode)
    res = run_bass_kernel_spmd(nc, in_maps, list(range(N_CORES)),
                               trace=trace)
    gather = gather_lite if mode == "lite" else gather_out
    return gather(res.results), res


def kernel(**inputs):
    inputs = {k: np.asarray(v) for k, v in inputs.items()}
    out, _ = run(inputs)
    return out



# revision 18
# speedup vs baseline: 2.2363x; 1.4062x over previous
"""BinaryFactoredLinear Trainium2 kernel.

Computes out = ((x * s2) @ sign(V)) @ sign(U).T * s1 + bias for
x [4, 4096, 4096] f32, factors [4096, 128] / [4096] — token-sharded
across 8 NeuronCores (2048 tokens each), run SPMD via
run_bass_kernel_spmd.

Host prep (exact f32 math, negligible vs HW time): x2 = x * s2 (same
op order as the reference), then x2 is split into xhi = bf16(x2) and
xlo = bf16(x2 - xhi) — together they carry ~16 mantissa bits, and the
sign matrices are +-1 so bf16 weights are exact. Each core's token
shard is pre-transposed and pre-tiled into contiguous [128, T] blocks
so every DMA is a contiguous 1 MiB transfer with the contraction dim
on SBUF partitions (no on-chip transposes, no on-chip dtype
conversions). The core writes its output transposed as contiguous
[nt, no, 128, T] blocks which the host reassembles.

Per-core pipeline (tokens tiled by T=512, all matmuls N=512 bf16):
  stage 1: z1T[r=128, T] += V_sign_k.T @ xhi_k + V_sign_k.T @ xlo_k
           (32 k-chunks accumulated in one PSUM bank)
  z1 split: DVE re-splits z1 (f32 PSUM) into bf16 hi/lo
  stage 2: outT[o*128:(o+1)*128, T] = U_sign_o @ [z1hi; z1lo]
  epilogue: ScalarE activation(Identity, scale=s1, bias=bias) — both
            per-partition APs — during the PSUM -> SBUF copy.

End-to-end rel err vs the f32 reference: ~3.5e-6 (HW-verified).
Other modes kept for experiments: f32 (exact, 4 cyc/row), f32r
(1 cyc/row, ~1.3e-4 on HW), bf16 (~2.4e-3), bf16x2 (on-chip hi/lo
split, same numerics as bf16x2h but extra ACT/DVE conversion load).
"""

import os
from contextlib import ExitStack

import numpy as np

import concourse.bacc as bacc
import concourse.mybir as mybir
import concourse.tile as tile
from concourse.bass_utils import run_bass_kernel_spmd

F32 = mybir.dt.float32
F32R = mybir.dt.float32r
BF16 = mybir.dt.bfloat16

B, S, D_IN, D_OUT, R = 4, 4096, 4096, 4096, 128
N_CORES = 8
TOKENS = B * S
TOK_PER_CORE = TOKENS // N_CORES

MODE = os.environ.get("BFL_MODE", "bf16x2h")
T_TILE = int(os.environ.get("BFL_T_TILE", "512"))
DMA_GROUP = int(os.environ.get("BFL_DMA_GROUP", "4"))
EPI = os.environ.get("BFL_EPI", "act")
LO_ENG = os.environ.get("BFL_LO_ENG", "dve")
XBUFS = int(os.environ.get("BFL_XBUFS", "5"))
LAYOUT = os.environ.get("BFL_LAYOUT", "std")


def build_nc(mode=MODE, d_in=D_IN, d_out=D_OUT, r=R, tok=TOK_PER_CORE,
             t_tile=T_TILE, loop=1, dma_group=DMA_GROUP, epi=EPI,
             lo_eng=LO_ENG, xbufs=XBUFS, layout=LAYOUT, probe="full",
             odma=os.environ.get("BFL_ODMA", "spread"), obufs=3, opbufs=4):
    if mode == "lite":
        return build_lite(tok=tok, d_in=d_in, d_out=d_out, r=r, loop=loop)
    assert d_in % 128 == 0 and d_out % 128 == 0 and tok % t_tile == 0
    assert r == 128 and t_tile <= 512
    nk, no, nt = d_in // 128, d_out // 128, tok // t_tile
    g = dma_group
    assert nk % g == 0 and no % g == 0

    if mode == "f32":
        xdt = wdt = F32
    elif mode == "f32r":
        xdt = wdt = F32R
    elif mode == "bf16x2h":
        xdt = wdt = BF16
    else:
        xdt, wdt = F32, BF16

    nc = bacc.Bacc("TRN2", target_bir_lowering=False, debug=False)

    if layout == "fat":
        xt = nc.dram_tensor("xt", [nt, nk // g, 128, g, t_tile], xdt,
                            kind="ExternalInput")
        outt = nc.dram_tensor("outt", [nt, no // g, 128, g, t_tile], F32,
                              kind="ExternalOutput")
    else:
        xt = nc.dram_tensor("xt", [nt, nk, 128, t_tile], xdt,
                            kind="ExternalInput")
        outt = nc.dram_tensor("outt", [nt, no, 128, t_tile], F32,
                              kind="ExternalOutput")
    if mode == "bf16x2h":
        assert layout == "std"
        xt2 = nc.dram_tensor("xt2", [nt, nk, 128, t_tile], BF16,
                             kind="ExternalInput")
    w1 = nc.dram_tensor("w1", [128, nk, r], wdt, kind="ExternalInput")
    w2 = nc.dram_tensor("w2", [r, d_out], wdt, kind="ExternalInput")
    s1c = nc.dram_tensor("s1c", [128, no], F32, kind="ExternalInput")
    biasc = nc.dram_tensor("biasc", [128, no], F32, kind="ExternalInput")

    Copy = mybir.ActivationFunctionType.Copy
    Ident = mybir.ActivationFunctionType.Identity
    sub = mybir.AluOpType.subtract
    mult = mybir.AluOpType.mult
    add = mybir.AluOpType.add
    lo_iface = nc.gpsimd if lo_eng == "pool" else nc.vector
    if odma == "spread":
        _rr = [0]

        def _dma():
            _rr[0] += 1
            return nc.sync if _rr[0] % 2 else nc.gpsimd
        in_dma = out_dma = lambda: _dma()
    else:
        out_iface = nc.gpsimd if odma == "pool" else nc.sync
        in_dma = lambda: nc.sync
        out_dma = lambda: out_iface

    with tile.TileContext(nc) as tc, ExitStack() as ctx:
        const = ctx.enter_context(tc.tile_pool(name="const", bufs=1))
        xpool = ctx.enter_context(tc.tile_pool(name="x", bufs=xbufs))
        z1s = ctx.enter_context(tc.tile_pool(name="z1s", bufs=2))
        osb = ctx.enter_context(tc.tile_pool(name="osb", bufs=obufs))
        z1pool = ctx.enter_context(
            tc.tile_pool(name="z1p", bufs=2, space="PSUM"))
        opsum = ctx.enter_context(
            tc.tile_pool(name="opsum", bufs=opbufs, space="PSUM"))
        if mode in ("bf16", "bf16x2"):
            hpool = ctx.enter_context(tc.tile_pool(name="hi", bufs=2 * xbufs))
        if mode == "bf16x2":
            lpool = ctx.enter_context(tc.tile_pool(name="lo", bufs=2 * xbufs))

        w1_sb = const.tile([128, nk, r], wdt)
        nc.sync.dma_start(w1_sb[:], w1.ap())
        w2_sb = const.tile([128, d_out], wdt)
        nc.sync.dma_start(w2_sb[:], w2.ap())
        s1_sb = const.tile([128, no], F32)
        nc.sync.dma_start(s1_sb[:], s1c.ap())
        b_sb = const.tile([128, no], F32)
        nc.sync.dma_start(b_sb[:], biasc.ap())

        if loop > 1:
            loop_cm = tc.For_i(
                0, loop, 1,
                hint_engines=(mybir.EngineType.PE, mybir.EngineType.DVE,
                              mybir.EngineType.Activation,
                              mybir.EngineType.SP))
            ctx.enter_context(loop_cm)

        for t in range(nt):
            z1p = z1pool.tile([128, t_tile], F32)
            xg, xg2 = {}, {}
            for kg in range(nk // g):
                xk = xpool.tile([128, g, t_tile], xdt)
                if probe != "nodma":
                    if layout == "fat":
                        in_dma().dma_start(xk[:], xt.ap()[t, kg])
                    else:
                        in_dma().dma_start(
                            xk[:], xt.ap()[t, kg * g:(kg + 1) * g].rearrange(
                                "g p s -> p g s"))
                xg[kg] = xk
                if mode == "bf16x2h":
                    xk2 = xpool.tile([128, g, t_tile], BF16, tag="xk2",
                                     name="xk2")
                    if probe != "nodma":
                        in_dma().dma_start(
                            xk2[:],
                            xt2.ap()[t, kg * g:(kg + 1) * g].rearrange(
                                "g p s -> p g s"))
                    xg2[kg] = xk2
            for k in range(nk):
                xk = xg[k // g][:, k % g, :]
                first, last = k == 0, k == nk - 1
                if mode == "bf16x2h":
                    xk2 = xg2[k // g][:, k % g, :]
                    nc.tensor.matmul(z1p[:], w1_sb[:, k, :], xk,
                                     start=first, stop=False)
                    nc.tensor.matmul(z1p[:], w1_sb[:, k, :], xk2,
                                     start=False, stop=last)
                elif mode in ("bf16", "bf16x2"):
                    hi = hpool.tile([128, t_tile], BF16)
                    nc.scalar.activation(hi[:], xk, Copy)
                    if mode == "bf16x2":
                        lo = lpool.tile([128, t_tile], BF16)
                        lo_iface.tensor_tensor(lo[:], xk, hi[:], sub)
                        nc.tensor.matmul(z1p[:], w1_sb[:, k, :], hi[:],
                                         start=first, stop=False)
                        nc.tensor.matmul(z1p[:], w1_sb[:, k, :], lo[:],
                                         start=False, stop=last)
                    else:
                        nc.tensor.matmul(z1p[:], w1_sb[:, k, :], hi[:],
                                         start=first, stop=last)
                else:
                    nc.tensor.matmul(z1p[:], w1_sb[:, k, :], xk,
                                     start=first, stop=last)

            if mode in ("bf16", "bf16x2", "bf16x2h"):
                z1hi = z1s.tile([128, t_tile], BF16, tag="z1hi")
                nc.vector.tensor_copy(z1hi[:], z1p[:])
                movers = [z1hi]
                if mode in ("bf16x2", "bf16x2h"):
                    z1lo = z1s.tile([128, t_tile], BF16, tag="z1lo")
                    nc.vector.tensor_tensor(z1lo[:], z1p[:], z1hi[:], sub)
                    movers.append(z1lo)
            else:
                z1f = z1s.tile([128, t_tile], xdt, tag="z1f")
                nc.vector.tensor_copy(z1f[:], z1p[:])
                movers = [z1f]

            for og in range(no // g):
                ob = osb.tile([128, g, t_tile], F32)
                for oi in range(g):
                    o = og * g + oi
                    op = opsum.tile([128, t_tile], F32)
                    for i, mv in enumerate(movers):
                        nc.tensor.matmul(
                            op[:], w2_sb[:, o * 128:(o + 1) * 128], mv[:],
                            start=(i == 0), stop=(i == len(movers) - 1))
                    if epi == "act":
                        nc.scalar.activation(ob[:, oi, :], op[:], Ident,
                                             bias=b_sb[:, o:o + 1],
                                             scale=s1_sb[:, o:o + 1])
                    else:
                        nc.vector.tensor_scalar(ob[:, oi, :], op[:],
                                                s1_sb[:, o:o + 1],
                                                b_sb[:, o:o + 1], mult, add)
                if probe != "nodma":
                    if layout == "fat":
                        out_dma().dma_start(outt.ap()[t, og], ob[:])
                    else:
                        out_dma().dma_start(
                            outt.ap()[t, og * g:(og + 1) * g].rearrange(
                                "g p s -> p g s"), ob[:])

    nc.compile()
    return nc


def build_lite(tok=TOK_PER_CORE, d_in=D_IN, d_out=D_OUT, r=R, loop=1,
               g_in=int(os.environ.get("BFL_GIN", "4")),
               g_out=int(os.environ.get("BFL_GOUT", "2")),
               xbufs=int(os.environ.get("BFL_LXBUFS", "3")),
               obufs=int(os.environ.get("BFL_LOBUFS", "3")),
               opbufs=4,
               in_eng=os.environ.get("BFL_LIN", "sync"),
               out_eng=os.environ.get("BFL_LOUT", "act"),
               lay=os.environ.get("BFL_LLAYOUT", "plane"),
               probe=os.environ.get("BFL_PROBE", "full"),
               epi2=os.environ.get("BFL_EPI2", "act")):
    """Memory-lean variant: bf16 x (s2 folded on host), bf16 sign(V),
    f32r stage-2, bf16 output. Per-core HBM traffic ~35 MB vs ~69 MB
    for bf16x2h. x is stored as [nk, 128, tok] k-planes so every input
    DMA is g_in contiguous 512 KiB planes; output is [no, 128, tok]
    o-planes, g_out planes per DMA."""
    assert d_in % 128 == 0 and d_out % 128 == 0 and r == 128
    nk, no, nt = d_in // 128, d_out // 128, tok // 512
    assert nk % g_in == 0 and no % g_out == 0

    nc = bacc.Bacc("TRN2", target_bir_lowering=False, debug=False)
    if lay == "fat":
        xt = nc.dram_tensor("xt", [nk // g_in, 128, g_in, tok], BF16,
                            kind="ExternalInput")
        outt = nc.dram_tensor("outt", [no // g_out, 128, g_out, tok], BF16,
                              kind="ExternalOutput")
    else:
        xt = nc.dram_tensor("xt", [nk, 128, tok], BF16,
                            kind="ExternalInput")
        outt = nc.dram_tensor("outt", [no, 128, tok], BF16,
                              kind="ExternalOutput")
    w1 = nc.dram_tensor("w1", [128, nk, r], BF16, kind="ExternalInput")
    w2 = nc.dram_tensor("w2", [r, d_out], F32R, kind="ExternalInput")
    s1c = nc.dram_tensor("s1c", [128, no], F32, kind="ExternalInput")
    biasc = nc.dram_tensor("biasc", [128, no], F32, kind="ExternalInput")

    Ident = mybir.ActivationFunctionType.Identity
    mult = mybir.AluOpType.mult
    add = mybir.AluOpType.add
    eng = {"sync": nc.sync, "act": nc.scalar, "pool": nc.gpsimd}
    in_dma, out_dma = eng[in_eng], eng[out_eng]

    with tile.TileContext(nc) as tc, ExitStack() as ctx:
        const = ctx.enter_context(tc.tile_pool(name="const", bufs=1))
        xpool = ctx.enter_context(tc.tile_pool(name="x", bufs=xbufs))
        z1s = ctx.enter_context(tc.tile_pool(name="z1s", bufs=2))
        osb = ctx.enter_context(tc.tile_pool(name="osb", bufs=obufs))
        z1pool = ctx.enter_context(
            tc.tile_pool(name="z1p", bufs=1, space="PSUM"))
        opsum = ctx.enter_context(
            tc.tile_pool(name="opsum", bufs=opbufs, space="PSUM"))

        w1_sb = const.tile([128, nk, r], BF16)
        nc.sync.dma_start(w1_sb[:], w1.ap())
        w2_sb = const.tile([128, d_out], F32R)
        nc.sync.dma_start(w2_sb[:], w2.ap())
        s1_sb = const.tile([128, no], F32)
        nc.sync.dma_start(s1_sb[:], s1c.ap())
        b_sb = const.tile([128, no], F32)
        nc.sync.dma_start(b_sb[:], biasc.ap())

        if loop > 1:
            loop_cm = tc.For_i(
                0, loop, 1,
                hint_engines=(mybir.EngineType.PE, mybir.EngineType.DVE,
                              mybir.EngineType.Activation,
                              mybir.EngineType.SP))
            ctx.enter_context(loop_cm)

        do_dma = probe != "nodma"
        do_compute = probe != "dmaonly"
        xg = {}
        for kg in range(nk // g_in):
            xk = xpool.tile([128, g_in, tok], BF16)
            if not do_dma:
                # probe: allocate via a tiny write so reads are legal
                nc.vector.tensor_copy(xk[:, 0, 0:16], s1_sb[:, 0:16])
            if do_dma:
                if lay == "fat":
                    in_dma.dma_start(xk[:], xt.ap()[kg])
                else:
                    in_dma.dma_start(
                        xk[:], xt.ap()[kg * g_in:(kg + 1) * g_in].rearrange(
                            "g p s -> p g s"))
            xg[kg] = xk
        z1p = [z1pool.tile([128, 512], F32, tag=f"z1p{t}", name=f"z1p{t}")
               for t in range(nt)]
        if do_compute:
            for k in range(nk):
                xk = xg[k // g_in][:, k % g_in, :]
                first, last = k == 0, k == nk - 1
                for t in range(nt):
                    nc.tensor.matmul(z1p[t][:], w1_sb[:, k, :],
                                     xk[:, t * 512:(t + 1) * 512],
                                     start=first, stop=last)
        z1r = []
        for t in range(nt):
            zr = z1s.tile([128, 512], F32R, tag=f"z1r{t}")
            if do_compute:
                nc.vector.tensor_copy(zr[:], z1p[t][:])
            z1r.append(zr)
        for og in range(no // g_out):
            ob = osb.tile([128, g_out, tok], BF16)
            if not do_compute:
                nc.vector.tensor_copy(ob[:, 0, 0:16], s1_sb[:, 0:16])
            if do_compute:
                for oi in range(g_out):
                    o = og * g_out + oi
                    for t in range(nt):
                        op = opsum.tile([128, 512], F32)
                        nc.tensor.matmul(op[:],
                                         w2_sb[:, o * 128:(o + 1) * 128],
                                         z1r[t][:], start=True, stop=True)
                        obs = ob[:, oi, t * 512:(t + 1) * 512]
                        use_dve = (epi2 == "dve"
                                   or (epi2 == "split" and t % 2 == 1))
                        if use_dve:
                            nc.vector.tensor_scalar(
                                obs, op[:], s1_sb[:, o:o + 1],
                                b_sb[:, o:o + 1], mult, add)
                        else:
                            nc.scalar.activation(obs, op[:], Ident,
                                                 bias=b_sb[:, o:o + 1],
                                                 scale=s1_sb[:, o:o + 1])
            if do_dma:
                if lay == "fat":
                    out_dma.dma_start(outt.ap()[og], ob[:])
                else:
                    out_dma.dma_start(
                        outt.ap()[og * g_out:(og + 1) * g_out].rearrange(
                            "g p s -> p g s"), ob[:])

    nc.compile()
    return nc


def prep_lite(x, U_latent, V_latent, s1, s2, bias, n_cores=N_CORES,
              g_in=int(os.environ.get("BFL_GIN", "4")),
              lay=os.environ.get("BFL_LLAYOUT", "plane")):
    import ml_dtypes

    tokens = x.shape[0] * x.shape[1] if x.ndim == 3 else x.shape[0]
    d_in = x.shape[-1]
    tok_pc = tokens // n_cores
    nk = d_in // 128

    x2 = x.reshape(tokens, d_in) * s2[None, :]
    xh = x2.astype(ml_dtypes.bfloat16)
    w1 = np.sign(V_latent).astype(np.float32)
    w1 = np.ascontiguousarray(
        w1.reshape(nk, 128, -1).transpose(1, 0, 2)).astype(
            ml_dtypes.bfloat16)
    # f32r has fp32 bit layout; pass the sign matrix as plain float32
    w2 = np.ascontiguousarray(np.sign(U_latent).astype(np.float32).T)
    no = w2.shape[1] // 128
    s1c = np.ascontiguousarray(s1.reshape(no, 128).T)
    biasc = np.ascontiguousarray(bias.reshape(no, 128).T)

    in_maps = []
    for c in range(n_cores):
        xs = xh[c * tok_pc:(c + 1) * tok_pc]
        if lay == "fat":
            # [nk//g, 128, g, tok]: one contiguous g*tok run per partition
            xtc = np.ascontiguousarray(
                xs.reshape(tok_pc, nk // g_in, g_in, 128).transpose(
                    1, 3, 2, 0))
        else:
            xtc = np.ascontiguousarray(
                xs.reshape(tok_pc, nk, 128).transpose(1, 2, 0))
        in_maps.append({"xt": xtc, "w1": w1, "w2": w2, "s1c": s1c,
                        "biasc": biasc})
    return in_maps


def gather_lite(results, n_cores=N_CORES,
                lay=os.environ.get("BFL_LLAYOUT", "plane")):
    out = np.empty((TOKENS, D_OUT), np.float32)
    for c in range(n_cores):
        ot = results[c]["outt"]
        if lay == "fat":
            # [no//g, 128, g, tok] -> [tok, d_out]
            shard = ot.transpose(3, 0, 2, 1).reshape(TOK_PER_CORE, D_OUT)
        else:
            # [no, 128, tok] -> [tok, d_out]
            shard = ot.transpose(2, 0, 1).reshape(TOK_PER_CORE, D_OUT)
        out[c * TOK_PER_CORE:(c + 1) * TOK_PER_CORE] = shard.astype(
            np.float32)
    return out.reshape(B, S, D_OUT)


def prep_inputs(x, U_latent, V_latent, s1, s2, bias, mode=MODE,
                n_cores=N_CORES, t_tile=T_TILE, layout=LAYOUT,
                dma_group=DMA_GROUP):
    if mode == "lite":
        return prep_lite(x, U_latent, V_latent, s1, s2, bias,
                         n_cores=n_cores)
    """Host-side prep: fold s2 into x, sign + cast factors, shard tokens."""
    import ml_dtypes

    tokens = x.shape[0] * x.shape[1] if x.ndim == 3 else x.shape[0]
    d_in = x.shape[-1]
    tok_pc = tokens // n_cores
    nt, nk = tok_pc // t_tile, d_in // 128
    g = dma_group

    x2 = x.reshape(tokens, d_in) * s2[None, :]
    w1 = np.sign(V_latent).astype(np.float32)
    # pack [d_in, r] -> [128, nk, r] so the SBUF upload is contiguous
    w1 = np.ascontiguousarray(
        w1.reshape(nk, 128, -1).transpose(1, 0, 2))
    w2 = np.ascontiguousarray(np.sign(U_latent).astype(np.float32).T)
    if mode in ("bf16", "bf16x2", "bf16x2h"):
        w1 = w1.astype(ml_dtypes.bfloat16)
        w2 = w2.astype(ml_dtypes.bfloat16)
    if mode == "bf16x2h":
        xhi = x2.astype(ml_dtypes.bfloat16)
        xlo = (x2 - xhi.astype(np.float32)).astype(ml_dtypes.bfloat16)
    no = w2.shape[1] // 128
    s1c = np.ascontiguousarray(s1.reshape(no, 128).T)
    biasc = np.ascontiguousarray(bias.reshape(no, 128).T)

    def tilefmt(arr2d, c):
        xs = arr2d[c * tok_pc:(c + 1) * tok_pc, :]
        if layout == "fat":
            # [nt, T, nk/g, g, 128] -> [nt, nk/g, 128, g, T]:
            # per partition a contiguous g*T run
            return np.ascontiguousarray(
                xs.reshape(nt, t_tile, nk // g, g, 128).transpose(
                    0, 2, 4, 3, 1))
        # [nt, T, nk, 128] -> [nt, nk, 128, T]
        return np.ascontiguousarray(
            xs.reshape(nt, t_tile, nk, 128).transpose(0, 2, 3, 1))

    in_maps = []
    for c in range(n_cores):
        m = {"w1": w1, "w2": w2, "s1c": s1c, "biasc": biasc}
        if mode == "bf16x2h":
            m["xt"] = tilefmt(xhi, c)
            m["xt2"] = tilefmt(xlo, c)
        else:
            m["xt"] = tilefmt(x2, c)
        in_maps.append(m)
    return in_maps


def gather_out(results, n_cores=N_CORES, t_tile=T_TILE, layout=LAYOUT,
               dma_group=DMA_GROUP):
    out = np.empty((TOKENS, D_OUT), np.float32)
    for c in range(n_cores):
        ot = results[c]["outt"]
        if layout == "fat":
            # [nt, no/g, 128, g, T] -> [tok_pc, d_out]
            shard = ot.transpose(0, 4, 1, 3, 2).reshape(TOK_PER_CORE, D_OUT)
        else:
            # [nt, no, 128, T] -> [tok_pc, d_out]
            shard = ot.transpose(0, 3, 1, 2).reshape(TOK_PER_CORE, D_OUT)
        out[c * TOK_PER_CORE:(c + 1) * TOK_PER_CORE, :] = shard
    return out.reshape(B, S, D_OUT)


_NC_CACHE = {}


def run(inputs, mode=MODE, trace=False):
    if mode not in _NC_CACHE:
        _NC_CACHE[mode] = build_nc(mode=mode)
    nc = _NC_CACHE[mode]
    in_maps = prep_inputs(**inputs, mode=mode)
    res = run_bass_kernel_spmd(nc, in_maps, list(range(N_CORES)),
                               trace=trace)
    gather = gather_lite if mode == "lite" else gather_out
    return gather(res.results), res


def kernel(**inputs):
    inputs = {k: np.asarray(v) for k, v in inputs.items()}
    out, _ = run(inputs)
    return out



# revision 20
# speedup vs baseline: 2.3938x; 1.0704x over previous
"""BinaryFactoredLinear Trainium2 kernel.

Computes out = ((x * s2) @ sign(V)) @ sign(U).T * s1 + bias for
x [4, 4096, 4096] f32, factors [4096, 128] / [4096] — token-sharded
across 8 NeuronCores (2048 tokens each), run SPMD via
run_bass_kernel_spmd.

Host prep (exact f32 math, negligible vs HW time): x2 = x * s2 (same
op order as the reference), then x2 is split into xhi = bf16(x2) and
xlo = bf16(x2 - xhi) — together they carry ~16 mantissa bits, and the
sign matrices are +-1 so bf16 weights are exact. Each core's token
shard is pre-transposed and pre-tiled into contiguous [128, T] blocks
so every DMA is a contiguous 1 MiB transfer with the contraction dim
on SBUF partitions (no on-chip transposes, no on-chip dtype
conversions). The core writes its output transposed as contiguous
[nt, no, 128, T] blocks which the host reassembles.

Per-core pipeline (tokens tiled by T=512, all matmuls N=512 bf16):
  stage 1: z1T[r=128, T] += V_sign_k.T @ xhi_k + V_sign_k.T @ xlo_k
           (32 k-chunks accumulated in one PSUM bank)
  z1 split: DVE re-splits z1 (f32 PSUM) into bf16 hi/lo
  stage 2: outT[o*128:(o+1)*128, T] = U_sign_o @ [z1hi; z1lo]
  epilogue: ScalarE activation(Identity, scale=s1, bias=bias) — both
            per-partition APs — during the PSUM -> SBUF copy.

End-to-end rel err vs the f32 reference: ~3.5e-6 (HW-verified).
Other modes kept for experiments: f32 (exact, 4 cyc/row), f32r
(1 cyc/row, ~1.3e-4 on HW), bf16 (~2.4e-3), bf16x2 (on-chip hi/lo
split, same numerics as bf16x2h but extra ACT/DVE conversion load).
"""

import os
from contextlib import ExitStack

import numpy as np

import concourse.bacc as bacc
import concourse.mybir as mybir
import concourse.tile as tile
from concourse.bass_utils import run_bass_kernel_spmd

F32 = mybir.dt.float32
F32R = mybir.dt.float32r
BF16 = mybir.dt.bfloat16

B, S, D_IN, D_OUT, R = 4, 4096, 4096, 4096, 128
N_CORES = 8
TOKENS = B * S
TOK_PER_CORE = TOKENS // N_CORES

MODE = os.environ.get("BFL_MODE", "bf16x2h")
T_TILE = int(os.environ.get("BFL_T_TILE", "512"))
DMA_GROUP = int(os.environ.get("BFL_DMA_GROUP", "4"))
EPI = os.environ.get("BFL_EPI", "act")
LO_ENG = os.environ.get("BFL_LO_ENG", "dve")
XBUFS = int(os.environ.get("BFL_XBUFS", "5"))
LAYOUT = os.environ.get("BFL_LAYOUT", "std")


def build_nc(mode=MODE, d_in=D_IN, d_out=D_OUT, r=R, tok=TOK_PER_CORE,
             t_tile=T_TILE, loop=1, dma_group=DMA_GROUP, epi=EPI,
             lo_eng=LO_ENG, xbufs=XBUFS, layout=LAYOUT, probe="full",
             odma=os.environ.get("BFL_ODMA", "spread"), obufs=3, opbufs=4):
    if mode == "lite":
        return build_lite(tok=tok, d_in=d_in, d_out=d_out, r=r, loop=loop)
    assert d_in % 128 == 0 and d_out % 128 == 0 and tok % t_tile == 0
    assert r == 128 and t_tile <= 512
    nk, no, nt = d_in // 128, d_out // 128, tok // t_tile
    g = dma_group
    assert nk % g == 0 and no % g == 0

    if mode == "f32":
        xdt = wdt = F32
    elif mode == "f32r":
        xdt = wdt = F32R
    elif mode == "bf16x2h":
        xdt = wdt = BF16
    else:
        xdt, wdt = F32, BF16

    nc = bacc.Bacc("TRN2", target_bir_lowering=False, debug=False)

    if layout == "fat":
        xt = nc.dram_tensor("xt", [nt, nk // g, 128, g, t_tile], xdt,
                            kind="ExternalInput")
        outt = nc.dram_tensor("outt", [nt, no // g, 128, g, t_tile], F32,
                              kind="ExternalOutput")
    else:
        xt = nc.dram_tensor("xt", [nt, nk, 128, t_tile], xdt,
                            kind="ExternalInput")
        outt = nc.dram_tensor("outt", [nt, no, 128, t_tile], F32,
                              kind="ExternalOutput")
    if mode == "bf16x2h":
        assert layout == "std"
        xt2 = nc.dram_tensor("xt2", [nt, nk, 128, t_tile], BF16,
                             kind="ExternalInput")
    w1 = nc.dram_tensor("w1", [128, nk, r], wdt, kind="ExternalInput")
    w2 = nc.dram_tensor("w2", [r, d_out], wdt, kind="ExternalInput")
    s1c = nc.dram_tensor("s1c", [128, no], F32, kind="ExternalInput")
    biasc = nc.dram_tensor("biasc", [128, no], F32, kind="ExternalInput")

    Copy = mybir.ActivationFunctionType.Copy
    Ident = mybir.ActivationFunctionType.Identity
    sub = mybir.AluOpType.subtract
    mult = mybir.AluOpType.mult
    add = mybir.AluOpType.add
    lo_iface = nc.gpsimd if lo_eng == "pool" else nc.vector
    if odma == "spread":
        _rr = [0]

        def _dma():
            _rr[0] += 1
            return nc.sync if _rr[0] % 2 else nc.gpsimd
        in_dma = out_dma = lambda: _dma()
    else:
        out_iface = nc.gpsimd if odma == "pool" else nc.sync
        in_dma = lambda: nc.sync
        out_dma = lambda: out_iface

    with tile.TileContext(nc) as tc, ExitStack() as ctx:
        const = ctx.enter_context(tc.tile_pool(name="const", bufs=1))
        xpool = ctx.enter_context(tc.tile_pool(name="x", bufs=xbufs))
        z1s = ctx.enter_context(tc.tile_pool(name="z1s", bufs=2))
        osb = ctx.enter_context(tc.tile_pool(name="osb", bufs=obufs))
        z1pool = ctx.enter_context(
            tc.tile_pool(name="z1p", bufs=2, space="PSUM"))
        opsum = ctx.enter_context(
            tc.tile_pool(name="opsum", bufs=opbufs, space="PSUM"))
        if mode in ("bf16", "bf16x2"):
            hpool = ctx.enter_context(tc.tile_pool(name="hi", bufs=2 * xbufs))
        if mode == "bf16x2":
            lpool = ctx.enter_context(tc.tile_pool(name="lo", bufs=2 * xbufs))

        w1_sb = const.tile([128, nk, r], wdt)
        nc.sync.dma_start(w1_sb[:], w1.ap())
        w2_sb = const.tile([128, d_out], wdt)
        nc.sync.dma_start(w2_sb[:], w2.ap())
        s1_sb = const.tile([128, no], F32)
        nc.sync.dma_start(s1_sb[:], s1c.ap())
        b_sb = const.tile([128, no], F32)
        nc.sync.dma_start(b_sb[:], biasc.ap())

        if loop > 1:
            loop_cm = tc.For_i(
                0, loop, 1,
                hint_engines=(mybir.EngineType.PE, mybir.EngineType.DVE,
                              mybir.EngineType.Activation,
                              mybir.EngineType.SP))
            ctx.enter_context(loop_cm)

        for t in range(nt):
            z1p = z1pool.tile([128, t_tile], F32)
            xg, xg2 = {}, {}
            for kg in range(nk // g):
                xk = xpool.tile([128, g, t_tile], xdt)
                if probe != "nodma":
                    if layout == "fat":
                        in_dma().dma_start(xk[:], xt.ap()[t, kg])
                    else:
                        in_dma().dma_start(
                            xk[:], xt.ap()[t, kg * g:(kg + 1) * g].rearrange(
                                "g p s -> p g s"))
                xg[kg] = xk
                if mode == "bf16x2h":
                    xk2 = xpool.tile([128, g, t_tile], BF16, tag="xk2",
                                     name="xk2")
                    if probe != "nodma":
                        in_dma().dma_start(
                            xk2[:],
                            xt2.ap()[t, kg * g:(kg + 1) * g].rearrange(
                                "g p s -> p g s"))
                    xg2[kg] = xk2
            for k in range(nk):
                xk = xg[k // g][:, k % g, :]
                first, last = k == 0, k == nk - 1
                if mode == "bf16x2h":
                    xk2 = xg2[k // g][:, k % g, :]
                    nc.tensor.matmul(z1p[:], w1_sb[:, k, :], xk,
                                     start=first, stop=False)
                    nc.tensor.matmul(z1p[:], w1_sb[:, k, :], xk2,
                                     start=False, stop=last)
                elif mode in ("bf16", "bf16x2"):
                    hi = hpool.tile([128, t_tile], BF16)
                    nc.scalar.activation(hi[:], xk, Copy)
                    if mode == "bf16x2":
                        lo = lpool.tile([128, t_tile], BF16)
                        lo_iface.tensor_tensor(lo[:], xk, hi[:], sub)
                        nc.tensor.matmul(z1p[:], w1_sb[:, k, :], hi[:],
                                         start=first, stop=False)
                        nc.tensor.matmul(z1p[:], w1_sb[:, k, :], lo[:],
                                         start=False, stop=last)
                    else:
                        nc.tensor.matmul(z1p[:], w1_sb[:, k, :], hi[:],
                                         start=first, stop=last)
                else:
                    nc.tensor.matmul(z1p[:], w1_sb[:, k, :], xk,
                                     start=first, stop=last)

            if mode in ("bf16", "bf16x2", "bf16x2h"):
                z1hi = z1s.tile([128, t_tile], BF16, tag="z1hi")
                nc.vector.tensor_copy(z1hi[:], z1p[:])
                movers = [z1hi]
                if mode in ("bf16x2", "bf16x2h"):
                    z1lo = z1s.tile([128, t_tile], BF16, tag="z1lo")
                    nc.vector.tensor_tensor(z1lo[:], z1p[:], z1hi[:], sub)
                    movers.append(z1lo)
            else:
                z1f = z1s.tile([128, t_tile], xdt, tag="z1f")
                nc.vector.tensor_copy(z1f[:], z1p[:])
                movers = [z1f]

            for og in range(no // g):
                ob = osb.tile([128, g, t_tile], F32)
                for oi in range(g):
                    o = og * g + oi
                    op = opsum.tile([128, t_tile], F32)
                    for i, mv in enumerate(movers):
                        nc.tensor.matmul(
                            op[:], w2_sb[:, o * 128:(o + 1) * 128], mv[:],
                            start=(i == 0), stop=(i == len(movers) - 1))
                    if epi == "act":
                        nc.scalar.activation(ob[:, oi, :], op[:], Ident,
                                             bias=b_sb[:, o:o + 1],
                                             scale=s1_sb[:, o:o + 1])
                    else:
                        nc.vector.tensor_scalar(ob[:, oi, :], op[:],
                                                s1_sb[:, o:o + 1],
                                                b_sb[:, o:o + 1], mult, add)
                if probe != "nodma":
                    if layout == "fat":
                        out_dma().dma_start(outt.ap()[t, og], ob[:])
                    else:
                        out_dma().dma_start(
                            outt.ap()[t, og * g:(og + 1) * g].rearrange(
                                "g p s -> p g s"), ob[:])

    nc.compile()
    return nc


def build_lite(tok=TOK_PER_CORE, d_in=D_IN, d_out=D_OUT, r=R, loop=1,
               g_in=int(os.environ.get("BFL_GIN", "4")),
               g_out=int(os.environ.get("BFL_GOUT", "2")),
               xbufs=int(os.environ.get("BFL_LXBUFS", "3")),
               obufs=int(os.environ.get("BFL_LOBUFS", "3")),
               opbufs=4,
               in_eng=os.environ.get("BFL_LIN", "sync"),
               out_eng=os.environ.get("BFL_LOUT", "act"),
               lay=os.environ.get("BFL_LLAYOUT", "plane"),
               probe=os.environ.get("BFL_PROBE", "full"),
               epi2=os.environ.get("BFL_EPI2", "act"),
               opw=int(os.environ.get("BFL_OPW", "512")),
               opbufs_env=os.environ.get("BFL_OPBUFS", "")):
    if opbufs_env:
        opbufs = int(opbufs_env)
    """Memory-lean variant: bf16 x (s2 folded on host), bf16 sign(V),
    f32r stage-2, bf16 output. Per-core HBM traffic ~35 MB vs ~69 MB
    for bf16x2h. x is stored as [nk, 128, tok] k-planes so every input
    DMA is g_in contiguous 512 KiB planes; output is [no, 128, tok]
    o-planes, g_out planes per DMA."""
    assert d_in % 128 == 0 and d_out % 128 == 0 and r == 128
    nk, no, nt = d_in // 128, d_out // 128, tok // 512
    assert nk % g_in == 0 and no % g_out == 0

    nc = bacc.Bacc("TRN2", target_bir_lowering=False, debug=False)
    if lay == "fat":
        xt = nc.dram_tensor("xt", [nk // g_in, 128, g_in, tok], BF16,
                            kind="ExternalInput")
        outt = nc.dram_tensor("outt", [no // g_out, 128, g_out, tok], BF16,
                              kind="ExternalOutput")
    else:
        xt = nc.dram_tensor("xt", [nk, 128, tok], BF16,
                            kind="ExternalInput")
        outt = nc.dram_tensor("outt", [no, 128, tok], BF16,
                              kind="ExternalOutput")
    w1 = nc.dram_tensor("w1", [128, nk, r], BF16, kind="ExternalInput")
    w2 = nc.dram_tensor("w2", [r, d_out], F32R, kind="ExternalInput")
    s1c = nc.dram_tensor("s1c", [128, no], F32, kind="ExternalInput")
    biasc = nc.dram_tensor("biasc", [128, no], F32, kind="ExternalInput")

    Ident = mybir.ActivationFunctionType.Identity
    mult = mybir.AluOpType.mult
    add = mybir.AluOpType.add
    eng = {"sync": nc.sync, "act": nc.scalar, "pool": nc.gpsimd}
    in_dma, out_dma = eng[in_eng], eng[out_eng]

    with tile.TileContext(nc) as tc, ExitStack() as ctx:
        const = ctx.enter_context(tc.tile_pool(name="const", bufs=1))
        xpool = ctx.enter_context(tc.tile_pool(name="x", bufs=xbufs))
        z1s = ctx.enter_context(tc.tile_pool(name="z1s", bufs=2))
        osb = ctx.enter_context(tc.tile_pool(name="osb", bufs=obufs))
        z1pool = ctx.enter_context(
            tc.tile_pool(name="z1p", bufs=1, space="PSUM"))
        opsum = ctx.enter_context(
            tc.tile_pool(name="opsum", bufs=opbufs, space="PSUM"))

        w1_sb = const.tile([128, nk, r], BF16)
        nc.sync.dma_start(w1_sb[:], w1.ap())
        w2_sb = const.tile([128, d_out], F32R)
        nc.sync.dma_start(w2_sb[:], w2.ap())
        s1_sb = const.tile([128, no], F32)
        nc.sync.dma_start(s1_sb[:], s1c.ap())
        b_sb = const.tile([128, no], F32)
        nc.sync.dma_start(b_sb[:], biasc.ap())

        if loop > 1:
            loop_cm = tc.For_i(
                0, loop, 1,
                hint_engines=(mybir.EngineType.PE, mybir.EngineType.DVE,
                              mybir.EngineType.Activation,
                              mybir.EngineType.SP))
            ctx.enter_context(loop_cm)

        do_dma = probe != "nodma"
        do_compute = probe != "dmaonly"
        xg = {}
        for kg in range(nk // g_in):
            xk = xpool.tile([128, g_in, tok], BF16)
            if not do_dma:
                # probe: allocate via a tiny write so reads are legal
                nc.vector.tensor_copy(xk[:, 0, 0:16], s1_sb[:, 0:16])
            if do_dma:
                if lay == "fat":
                    in_dma.dma_start(xk[:], xt.ap()[kg])
                else:
                    in_dma.dma_start(
                        xk[:], xt.ap()[kg * g_in:(kg + 1) * g_in].rearrange(
                            "g p s -> p g s"))
            xg[kg] = xk
        z1p = [z1pool.tile([128, 512], F32, tag=f"z1p{t}", name=f"z1p{t}")
               for t in range(nt)]
        if do_compute:
            for k in range(nk):
                xk = xg[k // g_in][:, k % g_in, :]
                first, last = k == 0, k == nk - 1
                for t in range(nt):
                    nc.tensor.matmul(z1p[t][:], w1_sb[:, k, :],
                                     xk[:, t * 512:(t + 1) * 512],
                                     start=first, stop=last)
        z1r = []
        for t in range(nt):
            zr = z1s.tile([128, 512], F32R, tag=f"z1r{t}")
            if do_compute:
                nc.vector.tensor_copy(zr[:], z1p[t][:])
            z1r.append(zr)
        for og in range(no // g_out):
            ob = osb.tile([128, g_out, tok], BF16)
            if not do_compute:
                nc.vector.tensor_copy(ob[:, 0, 0:16], s1_sb[:, 0:16])
            if do_compute:
                for oi in range(g_out):
                    o = og * g_out + oi
                    for i, t0 in enumerate(range(0, tok, opw)):
                        op = opsum.tile([128, opw], F32)
                        for j in range(opw // 512):
                            t = (t0 + j * 512) // 512
                            nc.tensor.matmul(op[:, j * 512:(j + 1) * 512],
                                             w2_sb[:,
                                                   o * 128:(o + 1) * 128],
                                             z1r[t][:], start=True,
                                             stop=True)
                        obs = ob[:, oi, t0:t0 + opw]
                        use_dve = (epi2 == "dve"
                                   or (epi2 == "split" and i % 2 == 1))
                        if use_dve:
                            nc.vector.tensor_scalar(
                                obs, op[:], s1_sb[:, o:o + 1],
                                b_sb[:, o:o + 1], mult, add)
                        else:
                            nc.scalar.activation(obs, op[:], Ident,
                                                 bias=b_sb[:, o:o + 1],
                                                 scale=s1_sb[:, o:o + 1])
            if do_dma:
                if lay == "fat":
                    out_dma.dma_start(outt.ap()[og], ob[:])
                else:
                    out_dma.dma_start(
                        outt.ap()[og * g_out:(og + 1) * g_out].rearrange(
                            "g p s -> p g s"), ob[:])

    nc.compile()
    return nc


def prep_lite(x, U_latent, V_latent, s1, s2, bias, n_cores=N_CORES,
              g_in=int(os.environ.get("BFL_GIN", "4")),
              lay=os.environ.get("BFL_LLAYOUT", "plane")):
    import ml_dtypes

    tokens = x.shape[0] * x.shape[1] if x.ndim == 3 else x.shape[0]
    d_in = x.shape[-1]
    tok_pc = tokens // n_cores
    nk = d_in // 128

    x2 = x.reshape(tokens, d_in) * s2[None, :]
    xh = x2.astype(ml_dtypes.bfloat16)
    w1 = np.sign(V_latent).astype(np.float32)
    w1 = np.ascontiguousarray(
        w1.reshape(nk, 128, -1).transpose(1, 0, 2)).astype(
            ml_dtypes.bfloat16)
    # f32r has fp32 bit layout; pass the sign matrix as plain float32
    w2 = np.ascontiguousarray(np.sign(U_latent).astype(np.float32).T)
    no = w2.shape[1] // 128
    s1c = np.ascontiguousarray(s1.reshape(no, 128).T)
    biasc = np.ascontiguousarray(bias.reshape(no, 128).T)

    in_maps = []
    for c in range(n_cores):
        xs = xh[c * tok_pc:(c + 1) * tok_pc]
        if lay == "fat":
            # [nk//g, 128, g, tok]: one contiguous g*tok run per partition
            xtc = np.ascontiguousarray(
                xs.reshape(tok_pc, nk // g_in, g_in, 128).transpose(
                    1, 3, 2, 0))
        else:
            xtc = np.ascontiguousarray(
                xs.reshape(tok_pc, nk, 128).transpose(1, 2, 0))
        in_maps.append({"xt": xtc, "w1": w1, "w2": w2, "s1c": s1c,
                        "biasc": biasc})
    return in_maps


def gather_lite(results, n_cores=N_CORES,
                lay=os.environ.get("BFL_LLAYOUT", "plane")):
    out = np.empty((TOKENS, D_OUT), np.float32)
    for c in range(n_cores):
        ot = results[c]["outt"]
        if lay == "fat":
            # [no//g, 128, g, tok] -> [tok, d_out]
            shard = ot.transpose(3, 0, 2, 1).reshape(TOK_PER_CORE, D_OUT)
        else:
            # [no, 128, tok] -> [tok, d_out]
            shard = ot.transpose(2, 0, 1).reshape(TOK_PER_CORE, D_OUT)
        out[c * TOK_PER_CORE:(c + 1) * TOK_PER_CORE] = shard.astype(
            np.float32)
    return out.reshape(B, S, D_OUT)


def prep_inputs(x, U_latent, V_latent, s1, s2, bias, mode=MODE,
                n_cores=N_CORES, t_tile=T_TILE, layout=LAYOUT,
                dma_group=DMA_GROUP):
    if mode == "lite":
        return prep_lite(x, U_latent, V_latent, s1, s2, bias,
                         n_cores=n_cores)
    """Host-side prep: fold s2 into x, sign + cast factors, shard tokens."""
    import ml_dtypes

    tokens = x.shape[0] * x.shape[1] if x.ndim == 3 else x.shape[0]
    d_in = x.shape[-1]
    tok_pc = tokens // n_cores
    nt, nk = tok_pc // t_tile, d_in // 128
    g = dma_group

    x2 = x.reshape(tokens, d_in) * s2[None, :]
    w1 = np.sign(V_latent).astype(np.float32)
    # pack [d_in, r] -> [128, nk, r] so the SBUF upload is contiguous
    w1 = np.ascontiguousarray(
        w1.reshape(nk, 128, -1).transpose(1, 0, 2))
    w2 = np.ascontiguousarray(np.sign(U_latent).astype(np.float32).T)
    if mode in ("bf16", "bf16x2", "bf16x2h"):
        w1 = w1.astype(ml_dtypes.bfloat16)
        w2 = w2.astype(ml_dtypes.bfloat16)
    if mode == "bf16x2h":
        xhi = x2.astype(ml_dtypes.bfloat16)
        xlo = (x2 - xhi.astype(np.float32)).astype(ml_dtypes.bfloat16)
    no = w2.shape[1] // 128
    s1c = np.ascontiguousarray(s1.reshape(no, 128).T)
    biasc = np.ascontiguousarray(bias.reshape(no, 128).T)

    def tilefmt(arr2d, c):
        xs = arr2d[c * tok_pc:(c + 1) * tok_pc, :]
        if layout == "fat":
            # [nt, T, nk/g, g, 128] -> [nt, nk/g, 128, g, T]:
            # per partition a contiguous g*T run
            return np.ascontiguousarray(
                xs.reshape(nt, t_tile, nk // g, g, 128).transpose(
                    0, 2, 4, 3, 1))
        # [nt, T, nk, 128] -> [nt, nk, 128, T]
        return np.ascontiguousarray(
            xs.reshape(nt, t_tile, nk, 128).transpose(0, 2, 3, 1))

    in_maps = []
    for c in range(n_cores):
        m = {"w1": w1, "w2": w2, "s1c": s1c, "biasc": biasc}
        if mode == "bf16x2h":
            m["xt"] = tilefmt(xhi, c)
            m["xt2"] = tilefmt(xlo, c)
        else:
            m["xt"] = tilefmt(x2, c)
        in_maps.append(m)
    return in_maps


def gather_out(results, n_cores=N_CORES, t_tile=T_TILE, layout=LAYOUT,
               dma_group=DMA_GROUP):
    out = np.empty((TOKENS, D_OUT), np.float32)
    for c in range(n_cores):
        ot = results[c]["outt"]
        if layout == "fat":
            # [nt, no/g, 128, g, T] -> [tok_pc, d_out]
            shard = ot.transpose(0, 4, 1, 3, 2).reshape(TOK_PER_CORE, D_OUT)
        else:
            # [nt, no, 128, T] -> [tok_pc, d_out]
            shard = ot.transpose(0, 3, 1, 2).reshape(TOK_PER_CORE, D_OUT)
        out[c * TOK_PER_CORE:(c + 1) * TOK_PER_CORE, :] = shard
    return out.reshape(B, S, D_OUT)


_NC_CACHE = {}


def run(inputs, mode=MODE, trace=False):
    if mode not in _NC_CACHE:
        _NC_CACHE[mode] = build_nc(mode=mode)
    nc = _NC_CACHE[mode]
    in_maps = prep_inputs(**inputs, mode=mode)
    res = run_bass_kernel_spmd(nc, in_maps, list(range(N_CORES)),
                               trace=trace)
    gather = gather_lite if mode == "lite" else gather_out
    return gather(res.results), res


def kernel(**inputs):
    inputs = {k: np.asarray(v) for k, v in inputs.items()}
    out, _ = run(inputs)
    return out



# revision 23
# speedup vs baseline: 2.4360x; 1.0176x over previous
"""BinaryFactoredLinear Trainium2 kernel.

Computes out = ((x * s2) @ sign(V)) @ sign(U).T * s1 + bias for
x [4, 4096, 4096] f32, factors [4096, 128] / [4096] — token-sharded
across 8 NeuronCores (2048 tokens each), run SPMD via
run_bass_kernel_spmd.

Host prep (exact f32 math, negligible vs HW time): x2 = x * s2 (same
op order as the reference), then x2 is split into xhi = bf16(x2) and
xlo = bf16(x2 - xhi) — together they carry ~16 mantissa bits, and the
sign matrices are +-1 so bf16 weights are exact. Each core's token
shard is pre-transposed and pre-tiled into contiguous [128, T] blocks
so every DMA is a contiguous 1 MiB transfer with the contraction dim
on SBUF partitions (no on-chip transposes, no on-chip dtype
conversions). The core writes its output transposed as contiguous
[nt, no, 128, T] blocks which the host reassembles.

Per-core pipeline (tokens tiled by T=512, all matmuls N=512 bf16):
  stage 1: z1T[r=128, T] += V_sign_k.T @ xhi_k + V_sign_k.T @ xlo_k
           (32 k-chunks accumulated in one PSUM bank)
  z1 split: DVE re-splits z1 (f32 PSUM) into bf16 hi/lo
  stage 2: outT[o*128:(o+1)*128, T] = U_sign_o @ [z1hi; z1lo]
  epilogue: ScalarE activation(Identity, scale=s1, bias=bias) — both
            per-partition APs — during the PSUM -> SBUF copy.

End-to-end rel err vs the f32 reference: ~3.5e-6 (HW-verified).
Other modes kept for experiments: f32 (exact, 4 cyc/row), f32r
(1 cyc/row, ~1.3e-4 on HW), bf16 (~2.4e-3), bf16x2 (on-chip hi/lo
split, same numerics as bf16x2h but extra ACT/DVE conversion load).
"""

import os
from contextlib import ExitStack

import numpy as np

import concourse.bacc as bacc
import concourse.mybir as mybir
import concourse.tile as tile
from concourse.bass_utils import run_bass_kernel_spmd

F32 = mybir.dt.float32
F32R = mybir.dt.float32r
BF16 = mybir.dt.bfloat16

B, S, D_IN, D_OUT, R = 4, 4096, 4096, 4096, 128
N_CORES = 8
TOKENS = B * S
TOK_PER_CORE = TOKENS // N_CORES

MODE = os.environ.get("BFL_MODE", "bf16x2h")
T_TILE = int(os.environ.get("BFL_T_TILE", "512"))
DMA_GROUP = int(os.environ.get("BFL_DMA_GROUP", "4"))
EPI = os.environ.get("BFL_EPI", "act")
LO_ENG = os.environ.get("BFL_LO_ENG", "dve")
XBUFS = int(os.environ.get("BFL_XBUFS", "5"))
LAYOUT = os.environ.get("BFL_LAYOUT", "std")


def build_nc(mode=MODE, d_in=D_IN, d_out=D_OUT, r=R, tok=TOK_PER_CORE,
             t_tile=T_TILE, loop=1, dma_group=DMA_GROUP, epi=EPI,
             lo_eng=LO_ENG, xbufs=XBUFS, layout=LAYOUT, probe="full",
             odma=os.environ.get("BFL_ODMA", "spread"), obufs=3, opbufs=4):
    if mode == "lite":
        return build_lite(tok=tok, d_in=d_in, d_out=d_out, r=r, loop=loop)
    assert d_in % 128 == 0 and d_out % 128 == 0 and tok % t_tile == 0
    assert r == 128 and t_tile <= 512
    nk, no, nt = d_in // 128, d_out // 128, tok // t_tile
    g = dma_group
    assert nk % g == 0 and no % g == 0

    if mode == "f32":
        xdt = wdt = F32
    elif mode == "f32r":
        xdt = wdt = F32R
    elif mode == "bf16x2h":
        xdt = wdt = BF16
    else:
        xdt, wdt = F32, BF16

    nc = bacc.Bacc("TRN2", target_bir_lowering=False, debug=False)

    if layout == "fat":
        xt = nc.dram_tensor("xt", [nt, nk // g, 128, g, t_tile], xdt,
                            kind="ExternalInput")
        outt = nc.dram_tensor("outt", [nt, no // g, 128, g, t_tile], F32,
                              kind="ExternalOutput")
    else:
        xt = nc.dram_tensor("xt", [nt, nk, 128, t_tile], xdt,
                            kind="ExternalInput")
        outt = nc.dram_tensor("outt", [nt, no, 128, t_tile], F32,
                              kind="ExternalOutput")
    if mode == "bf16x2h":
        assert layout == "std"
        xt2 = nc.dram_tensor("xt2", [nt, nk, 128, t_tile], BF16,
                             kind="ExternalInput")
    w1 = nc.dram_tensor("w1", [128, nk, r], wdt, kind="ExternalInput")
    w2 = nc.dram_tensor("w2", [r, d_out], wdt, kind="ExternalInput")
    s1c = nc.dram_tensor("s1c", [128, no], F32, kind="ExternalInput")
    biasc = nc.dram_tensor("biasc", [128, no], F32, kind="ExternalInput")

    Copy = mybir.ActivationFunctionType.Copy
    Ident = mybir.ActivationFunctionType.Identity
    sub = mybir.AluOpType.subtract
    mult = mybir.AluOpType.mult
    add = mybir.AluOpType.add
    lo_iface = nc.gpsimd if lo_eng == "pool" else nc.vector
    if odma == "spread":
        _rr = [0]

        def _dma():
            _rr[0] += 1
            return nc.sync if _rr[0] % 2 else nc.gpsimd
        in_dma = out_dma = lambda: _dma()
    else:
        out_iface = nc.gpsimd if odma == "pool" else nc.sync
        in_dma = lambda: nc.sync
        out_dma = lambda: out_iface

    with tile.TileContext(nc) as tc, ExitStack() as ctx:
        const = ctx.enter_context(tc.tile_pool(name="const", bufs=1))
        xpool = ctx.enter_context(tc.tile_pool(name="x", bufs=xbufs))
        z1s = ctx.enter_context(tc.tile_pool(name="z1s", bufs=2))
        osb = ctx.enter_context(tc.tile_pool(name="osb", bufs=obufs))
        z1pool = ctx.enter_context(
            tc.tile_pool(name="z1p", bufs=2, space="PSUM"))
        opsum = ctx.enter_context(
            tc.tile_pool(name="opsum", bufs=opbufs, space="PSUM"))
        if mode in ("bf16", "bf16x2"):
            hpool = ctx.enter_context(tc.tile_pool(name="hi", bufs=2 * xbufs))
        if mode == "bf16x2":
            lpool = ctx.enter_context(tc.tile_pool(name="lo", bufs=2 * xbufs))

        w1_sb = const.tile([128, nk, r], wdt)
        nc.sync.dma_start(w1_sb[:], w1.ap())
        w2_sb = const.tile([128, d_out], wdt)
        nc.sync.dma_start(w2_sb[:], w2.ap())
        s1_sb = const.tile([128, no], F32)
        nc.sync.dma_start(s1_sb[:], s1c.ap())
        b_sb = const.tile([128, no], F32)
        nc.sync.dma_start(b_sb[:], biasc.ap())

        if loop > 1:
            loop_cm = tc.For_i(
                0, loop, 1,
                hint_engines=(mybir.EngineType.PE, mybir.EngineType.DVE,
                              mybir.EngineType.Activation,
                              mybir.EngineType.SP))
            ctx.enter_context(loop_cm)

        for t in range(nt):
            z1p = z1pool.tile([128, t_tile], F32)
            xg, xg2 = {}, {}
            for kg in range(nk // g):
                xk = xpool.tile([128, g, t_tile], xdt)
                if probe != "nodma":
                    if layout == "fat":
                        in_dma().dma_start(xk[:], xt.ap()[t, kg])
                    else:
                        in_dma().dma_start(
                            xk[:], xt.ap()[t, kg * g:(kg + 1) * g].rearrange(
                                "g p s -> p g s"))
                xg[kg] = xk
                if mode == "bf16x2h":
                    xk2 = xpool.tile([128, g, t_tile], BF16, tag="xk2",
                                     name="xk2")
                    if probe != "nodma":
                        in_dma().dma_start(
                            xk2[:],
                            xt2.ap()[t, kg * g:(kg + 1) * g].rearrange(
                                "g p s -> p g s"))
                    xg2[kg] = xk2
            for k in range(nk):
                xk = xg[k // g][:, k % g, :]
                first, last = k == 0, k == nk - 1
                if mode == "bf16x2h":
                    xk2 = xg2[k // g][:, k % g, :]
                    nc.tensor.matmul(z1p[:], w1_sb[:, k, :], xk,
                                     start=first, stop=False)
                    nc.tensor.matmul(z1p[:], w1_sb[:, k, :], xk2,
                                     start=False, stop=last)
                elif mode in ("bf16", "bf16x2"):
                    hi = hpool.tile([128, t_tile], BF16)
                    nc.scalar.activation(hi[:], xk, Copy)
                    if mode == "bf16x2":
                        lo = lpool.tile([128, t_tile], BF16)
                        lo_iface.tensor_tensor(lo[:], xk, hi[:], sub)
                        nc.tensor.matmul(z1p[:], w1_sb[:, k, :], hi[:],
                                         start=first, stop=False)
                        nc.tensor.matmul(z1p[:], w1_sb[:, k, :], lo[:],
                                         start=False, stop=last)
                    else:
                        nc.tensor.matmul(z1p[:], w1_sb[:, k, :], hi[:],
                                         start=first, stop=last)
                else:
                    nc.tensor.matmul(z1p[:], w1_sb[:, k, :], xk,
                                     start=first, stop=last)

            if mode in ("bf16", "bf16x2", "bf16x2h"):
                z1hi = z1s.tile([128, t_tile], BF16, tag="z1hi")
                nc.vector.tensor_copy(z1hi[:], z1p[:])
                movers = [z1hi]
                if mode in ("bf16x2", "bf16x2h"):
                    z1lo = z1s.tile([128, t_tile], BF16, tag="z1lo")
                    nc.vector.tensor_tensor(z1lo[:], z1p[:], z1hi[:], sub)
                    movers.append(z1lo)
            else:
                z1f = z1s.tile([128, t_tile], xdt, tag="z1f")
                nc.vector.tensor_copy(z1f[:], z1p[:])
                movers = [z1f]

            for og in range(no // g):
                ob = osb.tile([128, g, t_tile], F32)
                for oi in range(g):
                    o = og * g + oi
                    op = opsum.tile([128, t_tile], F32)
                    for i, mv in enumerate(movers):
                        nc.tensor.matmul(
                            op[:], w2_sb[:, o * 128:(o + 1) * 128], mv[:],
                            start=(i == 0), stop=(i == len(movers) - 1))
                    if epi == "act":
                        nc.scalar.activation(ob[:, oi, :], op[:], Ident,
                                             bias=b_sb[:, o:o + 1],
                                             scale=s1_sb[:, o:o + 1])
                    else:
                        nc.vector.tensor_scalar(ob[:, oi, :], op[:],
                                                s1_sb[:, o:o + 1],
                                                b_sb[:, o:o + 1], mult, add)
                if probe != "nodma":
                    if layout == "fat":
                        out_dma().dma_start(outt.ap()[t, og], ob[:])
                    else:
                        out_dma().dma_start(
                            outt.ap()[t, og * g:(og + 1) * g].rearrange(
                                "g p s -> p g s"), ob[:])

    nc.compile()
    return nc


def build_lite(tok=TOK_PER_CORE, d_in=D_IN, d_out=D_OUT, r=R, loop=1,
               g_in=int(os.environ.get("BFL_GIN", "4")),
               g_out=int(os.environ.get("BFL_GOUT", "2")),
               xbufs=int(os.environ.get("BFL_LXBUFS", "3")),
               obufs=int(os.environ.get("BFL_LOBUFS", "3")),
               opbufs=4,
               in_eng=os.environ.get("BFL_LIN", "sync"),
               out_eng=os.environ.get("BFL_LOUT", "act"),
               lay=os.environ.get("BFL_LLAYOUT", "plane"),
               probe=os.environ.get("BFL_PROBE", "full"),
               epi2=os.environ.get("BFL_EPI2", "act"),
               opw=int(os.environ.get("BFL_OPW", "512")),
               opbufs_env=os.environ.get("BFL_OPBUFS", "")):
    if opbufs_env:
        opbufs = int(opbufs_env)
    """Memory-lean variant: bf16 x (s2 folded on host), bf16 sign(V),
    f32r stage-2, bf16 output. Per-core HBM traffic ~35 MB vs ~69 MB
    for bf16x2h. x is stored as [nk, 128, tok] k-planes so every input
    DMA is g_in contiguous 512 KiB planes; output is [no, 128, tok]
    o-planes, g_out planes per DMA."""
    assert d_in % 128 == 0 and d_out % 128 == 0 and r == 128
    nk, no, nt = d_in // 128, d_out // 128, tok // 512
    assert nk % g_in == 0 and no % g_out == 0

    nc = bacc.Bacc("TRN2", target_bir_lowering=False, debug=False)
    if lay == "fat":
        xt = nc.dram_tensor("xt", [nk // g_in, 128, g_in, tok], BF16,
                            kind="ExternalInput")
        outt = nc.dram_tensor("outt", [no // g_out, 128, g_out, tok], BF16,
                              kind="ExternalOutput")
    else:
        xt = nc.dram_tensor("xt", [nk, 128, tok], BF16,
                            kind="ExternalInput")
        outt = nc.dram_tensor("outt", [no, 128, tok], BF16,
                              kind="ExternalOutput")
    w1 = nc.dram_tensor("w1", [128, nk, r], BF16, kind="ExternalInput")
    w2 = nc.dram_tensor("w2", [r, d_out], F32R, kind="ExternalInput")
    s1c = nc.dram_tensor("s1c", [128, no], F32, kind="ExternalInput")
    biasc = nc.dram_tensor("biasc", [128, no], F32, kind="ExternalInput")

    Ident = mybir.ActivationFunctionType.Identity
    mult = mybir.AluOpType.mult
    add = mybir.AluOpType.add
    eng = {"sync": nc.sync, "act": nc.scalar, "pool": nc.gpsimd}
    in_dma, out_dma = eng[in_eng], eng[out_eng]

    with tile.TileContext(nc) as tc, ExitStack() as ctx:
        const = ctx.enter_context(tc.tile_pool(name="const", bufs=1))
        xpool = ctx.enter_context(tc.tile_pool(name="x", bufs=xbufs))
        z1s = ctx.enter_context(tc.tile_pool(name="z1s", bufs=2))
        osb = ctx.enter_context(tc.tile_pool(name="osb", bufs=obufs))
        z1pool = ctx.enter_context(
            tc.tile_pool(name="z1p", bufs=1, space="PSUM"))
        opsum = ctx.enter_context(
            tc.tile_pool(name="opsum", bufs=opbufs, space="PSUM"))

        w1_sb = const.tile([128, nk, r], BF16)
        nc.sync.dma_start(w1_sb[:], w1.ap())
        w2_sb = const.tile([128, d_out], F32R)
        nc.sync.dma_start(w2_sb[:], w2.ap())
        s1_sb = const.tile([128, no], F32)
        nc.sync.dma_start(s1_sb[:], s1c.ap())
        b_sb = const.tile([128, no], F32)
        nc.sync.dma_start(b_sb[:], biasc.ap())

        if loop > 1:
            loop_cm = tc.For_i(
                0, loop, 1,
                hint_engines=(mybir.EngineType.PE, mybir.EngineType.DVE,
                              mybir.EngineType.Activation,
                              mybir.EngineType.SP))
            ctx.enter_context(loop_cm)

        do_in = probe not in ("nodma", "dmaout")
        do_out = probe not in ("nodma", "dmain")
        do_compute = probe not in ("dmaonly", "dmain", "dmaout")
        xg = {}
        for kg in range(nk // g_in):
            if not (do_in or do_compute):
                continue
            xk = xpool.tile([128, g_in, tok], BF16)
            if do_compute and not do_in:
                # probe: allocate via a tiny write so reads are legal
                nc.vector.tensor_copy(xk[:, 0, 0:16], s1_sb[:, 0:16])
            if do_in:
                if lay == "fat":
                    in_dma.dma_start(xk[:], xt.ap()[kg])
                else:
                    in_dma.dma_start(
                        xk[:], xt.ap()[kg * g_in:(kg + 1) * g_in].rearrange(
                            "g p s -> p g s"))
            xg[kg] = xk
        z1p = [z1pool.tile([128, 512], F32, tag=f"z1p{t}", name=f"z1p{t}")
               for t in range(nt)]
        if do_compute:
            for k in range(nk):
                xk = xg[k // g_in][:, k % g_in, :]
                first, last = k == 0, k == nk - 1
                for t in range(nt):
                    nc.tensor.matmul(z1p[t][:], w1_sb[:, k, :],
                                     xk[:, t * 512:(t + 1) * 512],
                                     start=first, stop=last)
        z1r = []
        for t in range(nt):
            zr = z1s.tile([128, 512], F32R, tag=f"z1r{t}")
            if do_compute:
                nc.vector.tensor_copy(zr[:], z1p[t][:])
            z1r.append(zr)
        for og in range(no // g_out):
            if not (do_out or do_compute):
                continue
            ob = osb.tile([128, g_out, tok], BF16)
            if do_out and not do_compute:
                nc.vector.tensor_copy(ob[:, 0, 0:16], s1_sb[:, 0:16])
            if do_compute:
                for oi in range(g_out):
                    o = og * g_out + oi
                    for i, t0 in enumerate(range(0, tok, opw)):
                        op = opsum.tile([128, opw], F32)
                        for j in range(opw // 512):
                            t = (t0 + j * 512) // 512
                            nc.tensor.matmul(op[:, j * 512:(j + 1) * 512],
                                             w2_sb[:,
                                                   o * 128:(o + 1) * 128],
                                             z1r[t][:], start=True,
                                             stop=True)
                        obs = ob[:, oi, t0:t0 + opw]
                        use_dve = (epi2 == "dve"
                                   or (epi2 == "split" and i % 2 == 1)
                                   or (epi2 == "splitd" and i % 2 == 0))
                        if use_dve:
                            nc.vector.tensor_scalar(
                                obs, op[:], s1_sb[:, o:o + 1],
                                b_sb[:, o:o + 1], mult, add)
                        else:
                            nc.scalar.activation(obs, op[:], Ident,
                                                 bias=b_sb[:, o:o + 1],
                                                 scale=s1_sb[:, o:o + 1])
            if do_dma:
                if lay == "fat":
                    out_dma.dma_start(outt.ap()[og], ob[:])
                else:
                    out_dma.dma_start(
                        outt.ap()[og * g_out:(og + 1) * g_out].rearrange(
                            "g p s -> p g s"), ob[:])

    nc.compile()
    return nc


def prep_lite(x, U_latent, V_latent, s1, s2, bias, n_cores=N_CORES,
              g_in=int(os.environ.get("BFL_GIN", "4")),
              lay=os.environ.get("BFL_LLAYOUT", "plane")):
    import ml_dtypes

    tokens = x.shape[0] * x.shape[1] if x.ndim == 3 else x.shape[0]
    d_in = x.shape[-1]
    tok_pc = tokens // n_cores
    nk = d_in // 128

    x2 = x.reshape(tokens, d_in) * s2[None, :]
    xh = x2.astype(ml_dtypes.bfloat16)
    w1 = np.sign(V_latent).astype(np.float32)
    w1 = np.ascontiguousarray(
        w1.reshape(nk, 128, -1).transpose(1, 0, 2)).astype(
            ml_dtypes.bfloat16)
    # f32r has fp32 bit layout; pass the sign matrix as plain float32
    w2 = np.ascontiguousarray(np.sign(U_latent).astype(np.float32).T)
    no = w2.shape[1] // 128
    s1c = np.ascontiguousarray(s1.reshape(no, 128).T)
    biasc = np.ascontiguousarray(bias.reshape(no, 128).T)

    in_maps = []
    for c in range(n_cores):
        xs = xh[c * tok_pc:(c + 1) * tok_pc]
        if lay == "fat":
            # [nk//g, 128, g, tok]: one contiguous g*tok run per partition
            xtc = np.ascontiguousarray(
                xs.reshape(tok_pc, nk // g_in, g_in, 128).transpose(
                    1, 3, 2, 0))
        else:
            xtc = np.ascontiguousarray(
                xs.reshape(tok_pc, nk, 128).transpose(1, 2, 0))
        in_maps.append({"xt": xtc, "w1": w1, "w2": w2, "s1c": s1c,
                        "biasc": biasc})
    return in_maps


def gather_lite(results, n_cores=N_CORES,
                lay=os.environ.get("BFL_LLAYOUT", "plane")):
    out = np.empty((TOKENS, D_OUT), np.float32)
    for c in range(n_cores):
        ot = results[c]["outt"]
        if lay == "fat":
            # [no//g, 128, g, tok] -> [tok, d_out]
            shard = ot.transpose(3, 0, 2, 1).reshape(TOK_PER_CORE, D_OUT)
        else:
            # [no, 128, tok] -> [tok, d_out]
            shard = ot.transpose(2, 0, 1).reshape(TOK_PER_CORE, D_OUT)
        out[c * TOK_PER_CORE:(c + 1) * TOK_PER_CORE] = shard.astype(
            np.float32)
    return out.reshape(B, S, D_OUT)


def prep_inputs(x, U_latent, V_latent, s1, s2, bias, mode=MODE,
                n_cores=N_CORES, t_tile=T_TILE, layout=LAYOUT,
                dma_group=DMA_GROUP):
    if mode == "lite":
        return prep_lite(x, U_latent, V_latent, s1, s2, bias,
                         n_cores=n_cores)
    """Host-side prep: fold s2 into x, sign + cast factors, shard tokens."""
    import ml_dtypes

    tokens = x.shape[0] * x.shape[1] if x.ndim == 3 else x.shape[0]
    d_in = x.shape[-1]
    tok_pc = tokens // n_cores
    nt, nk = tok_pc // t_tile, d_in // 128
    g = dma_group

    x2 = x.reshape(tokens, d_in) * s2[None, :]
    w1 = np.sign(V_latent).astype(np.float32)
    # pack [d_in, r] -> [128, nk, r] so the SBUF upload is contiguous
    w1 = np.ascontiguousarray(
        w1.reshape(nk, 128, -1).transpose(1, 0, 2))
    w2 = np.ascontiguousarray(np.sign(U_latent).astype(np.float32).T)
    if mode in ("bf16", "bf16x2", "bf16x2h"):
        w1 = w1.astype(ml_dtypes.bfloat16)
        w2 = w2.astype(ml_dtypes.bfloat16)
    if mode == "bf16x2h":
        xhi = x2.astype(ml_dtypes.bfloat16)
        xlo = (x2 - xhi.astype(np.float32)).astype(ml_dtypes.bfloat16)
    no = w2.shape[1] // 128
    s1c = np.ascontiguousarray(s1.reshape(no, 128).T)
    biasc = np.ascontiguousarray(bias.reshape(no, 128).T)

    def tilefmt(arr2d, c):
        xs = arr2d[c * tok_pc:(c + 1) * tok_pc, :]
        if layout == "fat":
            # [nt, T, nk/g, g, 128] -> [nt, nk/g, 128, g, T]:
            # per partition a contiguous g*T run
            return np.ascontiguousarray(
                xs.reshape(nt, t_tile, nk // g, g, 128).transpose(
                    0, 2, 4, 3, 1))
        # [nt, T, nk, 128] -> [nt, nk, 128, T]
        return np.ascontiguousarray(
            xs.reshape(nt, t_tile, nk, 128).transpose(0, 2, 3, 1))

    in_maps = []
    for c in range(n_cores):
        m = {"w1": w1, "w2": w2, "s1c": s1c, "biasc": biasc}
        if mode == "bf16x2h":
            m["xt"] = tilefmt(xhi, c)
            m["xt2"] = tilefmt(xlo, c)
        else:
            m["xt"] = tilefmt(x2, c)
        in_maps.append(m)
    return in_maps


def gather_out(results, n_cores=N_CORES, t_tile=T_TILE, layout=LAYOUT,
               dma_group=DMA_GROUP):
    out = np.empty((TOKENS, D_OUT), np.float32)
    for c in range(n_cores):
        ot = results[c]["outt"]
        if layout == "fat":
            # [nt, no/g, 128, g, T] -> [tok_pc, d_out]
            shard = ot.transpose(0, 4, 1, 3, 2).reshape(TOK_PER_CORE, D_OUT)
        else:
            # [nt, no, 128, T] -> [tok_pc, d_out]
            shard = ot.transpose(0, 3, 1, 2).reshape(TOK_PER_CORE, D_OUT)
        out[c * TOK_PER_CORE:(c + 1) * TOK_PER_CORE, :] = shard
    return out.reshape(B, S, D_OUT)


_NC_CACHE = {}


def run(inputs, mode=MODE, trace=False):
    if mode not in _NC_CACHE:
        _NC_CACHE[mode] = build_nc(mode=mode)
    nc = _NC_CACHE[mode]
    in_maps = prep_inputs(**inputs, mode=mode)
    res = run_bass_kernel_spmd(nc, in_maps, list(range(N_CORES)),
                               trace=trace)
    gather = gather_lite if mode == "lite" else gather_out
    return gather(res.results), res


def kernel(**inputs):
    inputs = {k: np.asarray(v) for k, v in inputs.items()}
    out, _ = run(inputs)
    return out



# revision 24
# speedup vs baseline: 5.2585x; 2.1587x over previous
"""BinaryFactoredLinear Trainium2 kernel.

Computes out = ((x * s2) @ sign(V)) @ sign(U).T * s1 + bias for
x [4, 4096, 4096] f32, factors [4096, 128] / [4096] — token-sharded
across 8 NeuronCores (2048 tokens each), run SPMD via
run_bass_kernel_spmd.

Host prep (exact f32 math, negligible vs HW time): x2 = x * s2 (same
op order as the reference), then x2 is split into xhi = bf16(x2) and
xlo = bf16(x2 - xhi) — together they carry ~16 mantissa bits, and the
sign matrices are +-1 so bf16 weights are exact. Each core's token
shard is pre-transposed and pre-tiled into contiguous [128, T] blocks
so every DMA is a contiguous 1 MiB transfer with the contraction dim
on SBUF partitions (no on-chip transposes, no on-chip dtype
conversions). The core writes its output transposed as contiguous
[nt, no, 128, T] blocks which the host reassembles.

Per-core pipeline (tokens tiled by T=512, all matmuls N=512 bf16):
  stage 1: z1T[r=128, T] += V_sign_k.T @ xhi_k + V_sign_k.T @ xlo_k
           (32 k-chunks accumulated in one PSUM bank)
  z1 split: DVE re-splits z1 (f32 PSUM) into bf16 hi/lo
  stage 2: outT[o*128:(o+1)*128, T] = U_sign_o @ [z1hi; z1lo]
  epilogue: ScalarE activation(Identity, scale=s1, bias=bias) — both
            per-partition APs — during the PSUM -> SBUF copy.

End-to-end rel err vs the f32 reference: ~3.5e-6 (HW-verified).
Other modes kept for experiments: f32 (exact, 4 cyc/row), f32r
(1 cyc/row, ~1.3e-4 on HW), bf16 (~2.4e-3), bf16x2 (on-chip hi/lo
split, same numerics as bf16x2h but extra ACT/DVE conversion load).
"""

import os
from contextlib import ExitStack

import numpy as np

import concourse.bacc as bacc
import concourse.mybir as mybir
import concourse.tile as tile
from concourse.bass_utils import run_bass_kernel_spmd

F32 = mybir.dt.float32
F32R = mybir.dt.float32r
BF16 = mybir.dt.bfloat16

B, S, D_IN, D_OUT, R = 4, 4096, 4096, 4096, 128
N_CORES = 8
TOKENS = B * S
TOK_PER_CORE = TOKENS // N_CORES

MODE = os.environ.get("BFL_MODE", "bf16x2h")
T_TILE = int(os.environ.get("BFL_T_TILE", "512"))
DMA_GROUP = int(os.environ.get("BFL_DMA_GROUP", "4"))
EPI = os.environ.get("BFL_EPI", "act")
LO_ENG = os.environ.get("BFL_LO_ENG", "dve")
XBUFS = int(os.environ.get("BFL_XBUFS", "5"))
LAYOUT = os.environ.get("BFL_LAYOUT", "std")


def build_nc(mode=MODE, d_in=D_IN, d_out=D_OUT, r=R, tok=TOK_PER_CORE,
             t_tile=T_TILE, loop=1, dma_group=DMA_GROUP, epi=EPI,
             lo_eng=LO_ENG, xbufs=XBUFS, layout=LAYOUT, probe="full",
             odma=os.environ.get("BFL_ODMA", "spread"), obufs=3, opbufs=4):
    if mode == "lite":
        return build_lite(tok=tok, d_in=d_in, d_out=d_out, r=r, loop=loop)
    assert d_in % 128 == 0 and d_out % 128 == 0 and tok % t_tile == 0
    assert r == 128 and t_tile <= 512
    nk, no, nt = d_in // 128, d_out // 128, tok // t_tile
    g = dma_group
    assert nk % g == 0 and no % g == 0

    if mode == "f32":
        xdt = wdt = F32
    elif mode == "f32r":
        xdt = wdt = F32R
    elif mode == "bf16x2h":
        xdt = wdt = BF16
    else:
        xdt, wdt = F32, BF16

    nc = bacc.Bacc("TRN2", target_bir_lowering=False, debug=False)

    if layout == "fat":
        xt = nc.dram_tensor("xt", [nt, nk // g, 128, g, t_tile], xdt,
                            kind="ExternalInput")
        outt = nc.dram_tensor("outt", [nt, no // g, 128, g, t_tile], F32,
                              kind="ExternalOutput")
    else:
        xt = nc.dram_tensor("xt", [nt, nk, 128, t_tile], xdt,
                            kind="ExternalInput")
        outt = nc.dram_tensor("outt", [nt, no, 128, t_tile], F32,
                              kind="ExternalOutput")
    if mode == "bf16x2h":
        assert layout == "std"
        xt2 = nc.dram_tensor("xt2", [nt, nk, 128, t_tile], BF16,
                             kind="ExternalInput")
    w1 = nc.dram_tensor("w1", [128, nk, r], wdt, kind="ExternalInput")
    w2 = nc.dram_tensor("w2", [r, d_out], wdt, kind="ExternalInput")
    s1c = nc.dram_tensor("s1c", [128, no], F32, kind="ExternalInput")
    biasc = nc.dram_tensor("biasc", [128, no], F32, kind="ExternalInput")

    Copy = mybir.ActivationFunctionType.Copy
    Ident = mybir.ActivationFunctionType.Identity
    sub = mybir.AluOpType.subtract
    mult = mybir.AluOpType.mult
    add = mybir.AluOpType.add
    lo_iface = nc.gpsimd if lo_eng == "pool" else nc.vector
    if odma == "spread":
        _rr = [0]

        def _dma():
            _rr[0] += 1
            return nc.sync if _rr[0] % 2 else nc.gpsimd
        in_dma = out_dma = lambda: _dma()
    else:
        out_iface = nc.gpsimd if odma == "pool" else nc.sync
        in_dma = lambda: nc.sync
        out_dma = lambda: out_iface

    with tile.TileContext(nc) as tc, ExitStack() as ctx:
        const = ctx.enter_context(tc.tile_pool(name="const", bufs=1))
        xpool = ctx.enter_context(tc.tile_pool(name="x", bufs=xbufs))
        z1s = ctx.enter_context(tc.tile_pool(name="z1s", bufs=2))
        osb = ctx.enter_context(tc.tile_pool(name="osb", bufs=obufs))
        z1pool = ctx.enter_context(
            tc.tile_pool(name="z1p", bufs=2, space="PSUM"))
        opsum = ctx.enter_context(
            tc.tile_pool(name="opsum", bufs=opbufs, space="PSUM"))
        if mode in ("bf16", "bf16x2"):
            hpool = ctx.enter_context(tc.tile_pool(name="hi", bufs=2 * xbufs))
        if mode == "bf16x2":
            lpool = ctx.enter_context(tc.tile_pool(name="lo", bufs=2 * xbufs))

        w1_sb = const.tile([128, nk, r], wdt)
        nc.sync.dma_start(w1_sb[:], w1.ap())
        w2_sb = const.tile([128, d_out], wdt)
        nc.sync.dma_start(w2_sb[:], w2.ap())
        s1_sb = const.tile([128, no], F32)
        nc.sync.dma_start(s1_sb[:], s1c.ap())
        b_sb = const.tile([128, no], F32)
        nc.sync.dma_start(b_sb[:], biasc.ap())

        if loop > 1:
            loop_cm = tc.For_i(
                0, loop, 1,
                hint_engines=(mybir.EngineType.PE, mybir.EngineType.DVE,
                              mybir.EngineType.Activation,
                              mybir.EngineType.SP))
            ctx.enter_context(loop_cm)

        for t in range(nt):
            z1p = z1pool.tile([128, t_tile], F32)
            xg, xg2 = {}, {}
            for kg in range(nk // g):
                xk = xpool.tile([128, g, t_tile], xdt)
                if probe != "nodma":
                    if layout == "fat":
                        in_dma().dma_start(xk[:], xt.ap()[t, kg])
                    else:
                        in_dma().dma_start(
                            xk[:], xt.ap()[t, kg * g:(kg + 1) * g].rearrange(
                                "g p s -> p g s"))
                xg[kg] = xk
                if mode == "bf16x2h":
                    xk2 = xpool.tile([128, g, t_tile], BF16, tag="xk2",
                                     name="xk2")
                    if probe != "nodma":
                        in_dma().dma_start(
                            xk2[:],
                            xt2.ap()[t, kg * g:(kg + 1) * g].rearrange(
                                "g p s -> p g s"))
                    xg2[kg] = xk2
            for k in range(nk):
                xk = xg[k // g][:, k % g, :]
                first, last = k == 0, k == nk - 1
                if mode == "bf16x2h":
                    xk2 = xg2[k // g][:, k % g, :]
                    nc.tensor.matmul(z1p[:], w1_sb[:, k, :], xk,
                                     start=first, stop=False)
                    nc.tensor.matmul(z1p[:], w1_sb[:, k, :], xk2,
                                     start=False, stop=last)
                elif mode in ("bf16", "bf16x2"):
                    hi = hpool.tile([128, t_tile], BF16)
                    nc.scalar.activation(hi[:], xk, Copy)
                    if mode == "bf16x2":
                        lo = lpool.tile([128, t_tile], BF16)
                        lo_iface.tensor_tensor(lo[:], xk, hi[:], sub)
                        nc.tensor.matmul(z1p[:], w1_sb[:, k, :], hi[:],
                                         start=first, stop=False)
                        nc.tensor.matmul(z1p[:], w1_sb[:, k, :], lo[:],
                                         start=False, stop=last)
                    else:
                        nc.tensor.matmul(z1p[:], w1_sb[:, k, :], hi[:],
                                         start=first, stop=last)
                else:
                    nc.tensor.matmul(z1p[:], w1_sb[:, k, :], xk,
                                     start=first, stop=last)

            if mode in ("bf16", "bf16x2", "bf16x2h"):
                z1hi = z1s.tile([128, t_tile], BF16, tag="z1hi")
                nc.vector.tensor_copy(z1hi[:], z1p[:])
                movers = [z1hi]
                if mode in ("bf16x2", "bf16x2h"):
                    z1lo = z1s.tile([128, t_tile], BF16, tag="z1lo")
                    nc.vector.tensor_tensor(z1lo[:], z1p[:], z1hi[:], sub)
                    movers.append(z1lo)
            else:
                z1f = z1s.tile([128, t_tile], xdt, tag="z1f")
                nc.vector.tensor_copy(z1f[:], z1p[:])
                movers = [z1f]

            for og in range(no // g):
                ob = osb.tile([128, g, t_tile], F32)
                for oi in range(g):
                    o = og * g + oi
                    op = opsum.tile([128, t_tile], F32)
                    for i, mv in enumerate(movers):
                        nc.tensor.matmul(
                            op[:], w2_sb[:, o * 128:(o + 1) * 128], mv[:],
                            start=(i == 0), stop=(i == len(movers) - 1))
                    if epi == "act":
                        nc.scalar.activation(ob[:, oi, :], op[:], Ident,
                                             bias=b_sb[:, o:o + 1],
                                             scale=s1_sb[:, o:o + 1])
                    else:
                        nc.vector.tensor_scalar(ob[:, oi, :], op[:],
                                                s1_sb[:, o:o + 1],
                                                b_sb[:, o:o + 1], mult, add)
                if probe != "nodma":
                    if layout == "fat":
                        out_dma().dma_start(outt.ap()[t, og], ob[:])
                    else:
                        out_dma().dma_start(
                            outt.ap()[t, og * g:(og + 1) * g].rearrange(
                                "g p s -> p g s"), ob[:])

    nc.compile()
    return nc


def build_lite(tok=TOK_PER_CORE, d_in=D_IN, d_out=D_OUT, r=R, loop=1,
               g_in=int(os.environ.get("BFL_GIN", "4")),
               g_out=int(os.environ.get("BFL_GOUT", "2")),
               xbufs=int(os.environ.get("BFL_LXBUFS", "3")),
               obufs=int(os.environ.get("BFL_LOBUFS", "3")),
               opbufs=4,
               in_eng=os.environ.get("BFL_LIN", "sync"),
               out_eng=os.environ.get("BFL_LOUT", "act"),
               lay=os.environ.get("BFL_LLAYOUT", "plane"),
               probe=os.environ.get("BFL_PROBE", "full"),
               epi2=os.environ.get("BFL_EPI2", "act"),
               opw=int(os.environ.get("BFL_OPW", "512")),
               opbufs_env=os.environ.get("BFL_OPBUFS", "")):
    if opbufs_env:
        opbufs = int(opbufs_env)
    """Memory-lean variant: bf16 x (s2 folded on host), bf16 sign(V),
    f32r stage-2, bf16 output. Per-core HBM traffic ~35 MB vs ~69 MB
    for bf16x2h. x is stored as [nk, 128, tok] k-planes so every input
    DMA is g_in contiguous 512 KiB planes; output is [no, 128, tok]
    o-planes, g_out planes per DMA."""
    assert d_in % 128 == 0 and d_out % 128 == 0 and r == 128
    nk, no, nt = d_in // 128, d_out // 128, tok // 512
    assert nk % g_in == 0 and no % g_out == 0

    nc = bacc.Bacc("TRN2", target_bir_lowering=False, debug=False)
    if lay == "fat":
        xt = nc.dram_tensor("xt", [nk // g_in, 128, g_in, tok], BF16,
                            kind="ExternalInput")
        outt = nc.dram_tensor("outt", [no // g_out, 128, g_out, tok], BF16,
                              kind="ExternalOutput")
    else:
        xt = nc.dram_tensor("xt", [nk, 128, tok], BF16,
                            kind="ExternalInput")
        outt = nc.dram_tensor("outt", [no, 128, tok], BF16,
                              kind="ExternalOutput")
    w1 = nc.dram_tensor("w1", [128, nk, r], BF16, kind="ExternalInput")
    w2 = nc.dram_tensor("w2", [r, d_out], F32R, kind="ExternalInput")
    s1c = nc.dram_tensor("s1c", [128, no], F32, kind="ExternalInput")
    biasc = nc.dram_tensor("biasc", [128, no], F32, kind="ExternalInput")

    Ident = mybir.ActivationFunctionType.Identity
    mult = mybir.AluOpType.mult
    add = mybir.AluOpType.add
    eng = {"sync": nc.sync, "act": nc.scalar, "pool": nc.gpsimd}
    in_dma, out_dma = eng[in_eng], eng[out_eng]

    with tile.TileContext(nc) as tc, ExitStack() as ctx:
        const = ctx.enter_context(tc.tile_pool(name="const", bufs=1))
        xpool = ctx.enter_context(tc.tile_pool(name="x", bufs=xbufs))
        z1s = ctx.enter_context(tc.tile_pool(name="z1s", bufs=2))
        osb = ctx.enter_context(tc.tile_pool(name="osb", bufs=obufs))
        z1pool = ctx.enter_context(
            tc.tile_pool(name="z1p", bufs=1, space="PSUM"))
        opsum = ctx.enter_context(
            tc.tile_pool(name="opsum", bufs=opbufs, space="PSUM"))

        w1_sb = const.tile([128, nk, r], BF16)
        nc.sync.dma_start(w1_sb[:], w1.ap())
        w2_sb = const.tile([128, d_out], F32R)
        nc.sync.dma_start(w2_sb[:], w2.ap())
        s1_sb = const.tile([128, no], F32)
        nc.sync.dma_start(s1_sb[:], s1c.ap())
        b_sb = const.tile([128, no], F32)
        nc.sync.dma_start(b_sb[:], biasc.ap())

        if loop > 1:
            loop_cm = tc.For_i(
                0, loop, 1,
                hint_engines=(mybir.EngineType.PE, mybir.EngineType.DVE,
                              mybir.EngineType.Activation,
                              mybir.EngineType.SP))
            ctx.enter_context(loop_cm)

        do_in = probe not in ("nodma", "dmaout")
        do_out = probe not in ("nodma", "dmain")
        do_compute = probe not in ("dmaonly", "dmain", "dmaout")
        xg = {}
        for kg in range(nk // g_in):
            if not (do_in or do_compute):
                continue
            xk = xpool.tile([128, g_in, tok], BF16)
            if do_compute and not do_in:
                # probe: allocate via a tiny write so reads are legal
                nc.vector.tensor_copy(xk[:, 0, 0:16], s1_sb[:, 0:16])
            if do_in:
                if lay == "fat":
                    in_dma.dma_start(xk[:], xt.ap()[kg])
                else:
                    in_dma.dma_start(
                        xk[:], xt.ap()[kg * g_in:(kg + 1) * g_in].rearrange(
                            "g p s -> p g s"))
            xg[kg] = xk
        z1p = [z1pool.tile([128, 512], F32, tag=f"z1p{t}", name=f"z1p{t}")
               for t in range(nt)]
        if do_compute:
            for k in range(nk):
                xk = xg[k // g_in][:, k % g_in, :]
                first, last = k == 0, k == nk - 1
                for t in range(nt):
                    nc.tensor.matmul(z1p[t][:], w1_sb[:, k, :],
                                     xk[:, t * 512:(t + 1) * 512],
                                     start=first, stop=last)
        z1r = []
        for t in range(nt):
            zr = z1s.tile([128, 512], F32R, tag=f"z1r{t}")
            if do_compute:
                nc.vector.tensor_copy(zr[:], z1p[t][:])
            z1r.append(zr)
        for og in range(no // g_out):
            if not (do_out or do_compute):
                continue
            ob = osb.tile([128, g_out, tok], BF16)
            if do_out and not do_compute:
                nc.vector.tensor_copy(ob[:, 0, 0:16], s1_sb[:, 0:16])
            if do_compute:
                for oi in range(g_out):
                    o = og * g_out + oi
                    for i, t0 in enumerate(range(0, tok, opw)):
                        op = opsum.tile([128, opw], F32)
                        for j in range(opw // 512):
                            t = (t0 + j * 512) // 512
                            nc.tensor.matmul(op[:, j * 512:(j + 1) * 512],
                                             w2_sb[:,
                                                   o * 128:(o + 1) * 128],
                                             z1r[t][:], start=True,
                                             stop=True)
                        obs = ob[:, oi, t0:t0 + opw]
                        use_dve = (epi2 == "dve"
                                   or (epi2 == "split" and i % 2 == 1)
                                   or (epi2 == "splitd" and i % 2 == 0))
                        if use_dve:
                            nc.vector.tensor_scalar(
                                obs, op[:], s1_sb[:, o:o + 1],
                                b_sb[:, o:o + 1], mult, add)
                        else:
                            nc.scalar.activation(obs, op[:], Ident,
                                                 bias=b_sb[:, o:o + 1],
                                                 scale=s1_sb[:, o:o + 1])
            if do_out:
                if lay == "fat":
                    out_dma.dma_start(outt.ap()[og], ob[:])
                else:
                    out_dma.dma_start(
                        outt.ap()[og * g_out:(og + 1) * g_out].rearrange(
                            "g p s -> p g s"), ob[:])

    nc.compile()
    return nc


def prep_lite(x, U_latent, V_latent, s1, s2, bias, n_cores=N_CORES,
              g_in=int(os.environ.get("BFL_GIN", "4")),
              lay=os.environ.get("BFL_LLAYOUT", "plane")):
    import ml_dtypes

    tokens = x.shape[0] * x.shape[1] if x.ndim == 3 else x.shape[0]
    d_in = x.shape[-1]
    tok_pc = tokens // n_cores
    nk = d_in // 128

    x2 = x.reshape(tokens, d_in) * s2[None, :]
    xh = x2.astype(ml_dtypes.bfloat16)
    w1 = np.sign(V_latent).astype(np.float32)
    w1 = np.ascontiguousarray(
        w1.reshape(nk, 128, -1).transpose(1, 0, 2)).astype(
            ml_dtypes.bfloat16)
    # f32r has fp32 bit layout; pass the sign matrix as plain float32
    w2 = np.ascontiguousarray(np.sign(U_latent).astype(np.float32).T)
    no = w2.shape[1] // 128
    s1c = np.ascontiguousarray(s1.reshape(no, 128).T)
    biasc = np.ascontiguousarray(bias.reshape(no, 128).T)

    in_maps = []
    for c in range(n_cores):
        xs = xh[c * tok_pc:(c + 1) * tok_pc]
        if lay == "fat":
            # [nk//g, 128, g, tok]: one contiguous g*tok run per partition
            xtc = np.ascontiguousarray(
                xs.reshape(tok_pc, nk // g_in, g_in, 128).transpose(
                    1, 3, 2, 0))
        else:
            xtc = np.ascontiguousarray(
                xs.reshape(tok_pc, nk, 128).transpose(1, 2, 0))
        in_maps.append({"xt": xtc, "w1": w1, "w2": w2, "s1c": s1c,
                        "biasc": biasc})
    return in_maps


def gather_lite(results, n_cores=N_CORES,
                lay=os.environ.get("BFL_LLAYOUT", "plane")):
    out = np.empty((TOKENS, D_OUT), np.float32)
    for c in range(n_cores):
        ot = results[c]["outt"]
        if lay == "fat":
            # [no//g, 128, g, tok] -> [tok, d_out]
            shard = ot.transpose(3, 0, 2, 1).reshape(TOK_PER_CORE, D_OUT)
        else:
            # [no, 128, tok] -> [tok, d_out]
            shard = ot.transpose(2, 0, 1).reshape(TOK_PER_CORE, D_OUT)
        out[c * TOK_PER_CORE:(c + 1) * TOK_PER_CORE] = shard.astype(
            np.float32)
    return out.reshape(B, S, D_OUT)


def prep_inputs(x, U_latent, V_latent, s1, s2, bias, mode=MODE,
                n_cores=N_CORES, t_tile=T_TILE, layout=LAYOUT,
                dma_group=DMA_GROUP):
    if mode == "lite":
        return prep_lite(x, U_latent, V_latent, s1, s2, bias,
                         n_cores=n_cores)
    """Host-side prep: fold s2 into x, sign + cast factors, shard tokens."""
    import ml_dtypes

    tokens = x.shape[0] * x.shape[1] if x.ndim == 3 else x.shape[0]
    d_in = x.shape[-1]
    tok_pc = tokens // n_cores
    nt, nk = tok_pc // t_tile, d_in // 128
    g = dma_group

    x2 = x.reshape(tokens, d_in) * s2[None, :]
    w1 = np.sign(V_latent).astype(np.float32)
    # pack [d_in, r] -> [128, nk, r] so the SBUF upload is contiguous
    w1 = np.ascontiguousarray(
        w1.reshape(nk, 128, -1).transpose(1, 0, 2))
    w2 = np.ascontiguousarray(np.sign(U_latent).astype(np.float32).T)
    if mode in ("bf16", "bf16x2", "bf16x2h"):
        w1 = w1.astype(ml_dtypes.bfloat16)
        w2 = w2.astype(ml_dtypes.bfloat16)
    if mode == "bf16x2h":
        xhi = x2.astype(ml_dtypes.bfloat16)
        xlo = (x2 - xhi.astype(np.float32)).astype(ml_dtypes.bfloat16)
    no = w2.shape[1] // 128
    s1c = np.ascontiguousarray(s1.reshape(no, 128).T)
    biasc = np.ascontiguousarray(bias.reshape(no, 128).T)

    def tilefmt(arr2d, c):
        xs = arr2d[c * tok_pc:(c + 1) * tok_pc, :]
        if layout == "fat":
            # [nt, T, nk/g, g, 128] -> [nt, nk/g, 128, g, T]:
            # per partition a contiguous g*T run
            return np.ascontiguousarray(
                xs.reshape(nt, t_tile, nk // g, g, 128).transpose(
                    0, 2, 4, 3, 1))
        # [nt, T, nk, 128] -> [nt, nk, 128, T]
        return np.ascontiguousarray(
            xs.reshape(nt, t_tile, nk, 128).transpose(0, 2, 3, 1))

    in_maps = []
    for c in range(n_cores):
        m = {"w1": w1, "w2": w2, "s1c": s1c, "biasc": biasc}
        if mode == "bf16x2h":
            m["xt"] = tilefmt(xhi, c)
            m["xt2"] = tilefmt(xlo, c)
        else:
            m["xt"] = tilefmt(x2, c)
        in_maps.append(m)
    return in_maps


def gather_out(results, n_cores=N_CORES, t_tile=T_TILE, layout=LAYOUT,
               dma_group=DMA_GROUP):
    out = np.empty((TOKENS, D_OUT), np.float32)
    for c in range(n_cores):
        ot = results[c]["outt"]
        if layout == "fat":
            # [nt, no/g, 128, g, T] -> [tok_pc, d_out]
            shard = ot.transpose(0, 4, 1, 3, 2).reshape(TOK_PER_CORE, D_OUT)
        else:
            # [nt, no, 128, T] -> [tok_pc, d_out]
            shard = ot.transpose(0, 3, 1, 2).reshape(TOK_PER_CORE, D_OUT)
        out[c * TOK_PER_CORE:(c + 1) * TOK_PER_CORE, :] = shard
    return out.reshape(B, S, D_OUT)


_NC_CACHE = {}


def run(inputs, mode=MODE, trace=False):
    if mode not in _NC_CACHE:
        _NC_CACHE[mode] = build_nc(mode=mode)
    nc = _NC_CACHE[mode]
    in_maps = prep_inputs(**inputs, mode=mode)
    res = run_bass_kernel_spmd(nc, in_maps, list(range(N_CORES)),
                               trace=trace)
    gather = gather_lite if mode == "lite" else gather_out
    return gather(res.results), res


def kernel(**inputs):
    inputs = {k: np.asarray(v) for k, v in inputs.items()}
    out, _ = run(inputs)
    return out

